# revision 27
# baseline (speedup 1.0000x reference)
"""GAT (2-layer, 4->1 heads) + global mean pool + classifier on 8 trn2 NeuronCores.

Sharding: nodes (and their incoming edges) partitioned contiguously across the
8 cores; small weights replicated; per-layer node-feature tables AllGathered
(chunked, overlapped with producer compute); per-graph pooled sums AllReduced.

Layer-1 table rows are rotated per head (first basis vector = att_src) so the
512B row carries both the message features and alpha_src; the inverse rotation
is folded into the transposed epilogue feeding the layer-2 head. The edge
one-hot transpose (for the alpha_dst lookup) is done on the PE instead of a
broadcast DMA + compare.

Self-contained: takes full inputs, returns full [64, 2] log-softmax output.
"""
import sys
for _p in ('/opt/trn_rl_repo', '/root/.axon_site/_ro/trn_rl_repo'):
    if _p not in sys.path:
        sys.path.insert(0, _p)

import numpy as np
import concourse.bass as bass
import concourse.bacc as bacc
import concourse.tile as tile
import concourse.mybir as mybir
from concourse import bass_utils, library_config

dt = mybir.dt

# problem constants (hardcoded per contract); N/E re-derivable for sim tests
N = 50000
E = 1600000
G = 64
DIN = 128
HID = 64
H = 4
NEG_SLOPE = 0.2
BN_EPS = 1e-5
NC = 8
RF1 = 256                # L1 table row: fp16 slots (512B): y = xw @ Q (as = y[h,0])
RF2 = 128                # L2 table row: fp16 slots (256B): 64 xws | 2 a_src(f32) | pad


def configure(n, e):
    global N, E, NSH, NT, NPAD, HALF
    N, E = n, e
    NSH = N // NC
    NT = (NSH + 127) // 128
    NPAD = NT * 128
    HALF = NC // 2 * NPAD


configure(N, E)

_cache = {}


def _prep_host(x, edge_index, batch,
               W1, att_src1, att_dst1, bias1, bn1_g, bn1_b, bn1_m, bn1_v,
               W2, att_src2, att_dst2, bias2, bn2_g, bn2_b, bn2_m, bn2_v,
               Wc1, bc1, Wc2, bc2):
    """Index-space layout + folded weights. Returns (in_maps, CH, meta)."""
    f32 = np.float32
    src = np.concatenate([np.asarray(edge_index[0], np.int64),
                          np.arange(N, dtype=np.int64)])
    dst = np.concatenate([np.asarray(edge_index[1], np.int64),
                          np.arange(N, dtype=np.int64)])
    EE = src.shape[0]

    # relabel nodes: snake-assign by in-degree so every (core, tile) bucket
    # has a near-equal edge count (balances chunk counts across cores)
    indeg = np.bincount(dst, minlength=N)
    order_by_deg = np.argsort(-indeg, kind='stable')
    nbuckets = NC * NT
    newid = np.empty(N, np.int64)
    bidx = np.arange(N) % nbuckets
    snake = (np.arange(N) // nbuckets) % 2 == 1
    bidx = np.where(snake, nbuckets - 1 - bidx, bidx)
    # bucket b corresponds to core b // NT, tile b % NT
    slot_in_bucket = np.zeros(N, np.int64)
    counts = np.zeros(nbuckets, np.int64)
    for i in range(N):
        b = bidx[i]
        while counts[b] >= 128:
            b = (b + 1) % nbuckets
        slot_in_bucket[i] = counts[b]
        counts[b] += 1
        bidx[i] = b
    cores_of = bidx // NT
    tiles_of = bidx % NT
    newid[order_by_deg] = cores_of * NSH + tiles_of * 128 + slot_in_bucket
    # tile NT-1 slots beyond NSH are ghosts; ensure none assigned
    lastcap = NSH - (NT - 1) * 128
    bad = (tiles_of == NT - 1) & (slot_in_bucket >= lastcap)
    if bad.any():
        ov = np.where(bad)[0]
        free_buckets = [b for b in range(nbuckets)
                        if (b % NT != NT - 1 and counts[b] < 128)
                        or (b % NT == NT - 1 and counts[b] < lastcap)]
        fi = 0
        for i in ov:
            while True:
                b = free_buckets[fi % len(free_buckets)]
                cap = 128 if b % NT != NT - 1 else lastcap
                if counts[b] < cap:
                    break
                fi += 1
            slot_in_bucket[i] = counts[b]
            counts[b] += 1
            cores_of[i] = b // NT
            tiles_of[i] = b % NT
            fi += 1
        newid[order_by_deg] = cores_of * NSH + tiles_of * 128 + slot_in_bucket
    inv = np.empty(N, np.int64)
    inv[newid] = np.arange(N)      # inv[new] = old
    src = newid[src]
    dst = newid[dst]

    core = dst // NSH
    ldst = dst - core * NSH
    t = ldst >> 7                     # dst tile within shard
    dit = ldst & 127                  # dst index within tile
    # table rows laid out chunk-major (per allgather chunk, core-major
    # inside) so each chunk's collective output is contiguous
    tA = NT // 2
    rowsA = tA * 128
    rowsB = NPAD - rowsA
    ls = src % NSH
    scA = ls < rowsA
    trow = np.where(scA, (src // NSH) * rowsA + ls,
                    NC * rowsA + (src // NSH) * rowsB + (ls - rowsA))
    g = (trow >= HALF).astype(np.int64)           # table half by row
    lidx = trow - g * HALF                        # int16-safe local row

    key = ((core * NT + t) * 2 + g)   # bucket id, core-major
    nbuck = NC * NT * 2
    cnt = np.bincount(key, minlength=nbuck).reshape(NC, NT, 2)
    CH = np.maximum(1, (cnt.max(axis=0) + 127) // 128)   # [NT, 2] shared chunks
    CH = CH + (CH & 1)   # even counts -> 4B-aligned stream slice offsets
    CHUNKS = int(CH.sum())
    choff = np.zeros((NT, 2), np.int64)
    choff.reshape(-1)[1:] = np.cumsum(CH.reshape(-1))[:-1]

    # stable-sort edges by bucket; ranks within bucket
    order = np.argsort(key, kind='stable')
    skey = key[order]
    bstart = np.searchsorted(skey, np.arange(nbuck))
    rank = np.arange(EE, dtype=np.int64) - bstart[skey]
    # padded stream position (per core stream of CHUNKS*128 slots)
    bt = (skey // 2) % NT
    bg = skey % 2
    pos = choff[bt, bg] * 128 + rank
    scor = skey // (NT * 2)

    gidx_all = np.zeros((NC, CHUNKS * 128), np.int16)
    dcol_all = np.full((NC, CHUNKS * 128), 999.0, np.float16)
    for c in range(NC):
        m = scor == c
        eidx = order[m]
        gidx_all[c, pos[m]] = lidx[eidx].astype(np.int16)
        dcol_all[c, pos[m]] = dit[eidx].astype(np.float16)

    # wrap: element i -> [i % 16, i // 16] / dstcol: chunk-major -> [128, CHUNKS]
    gidx = np.tile(gidx_all.reshape(NC, CHUNKS * 8, 16).transpose(0, 2, 1), (1, 8, 1)).copy()
    dcol = dcol_all.reshape(NC, CHUNKS, 128).transpose(0, 2, 1).copy()

    # batch / pooling (note: node n' holds old node inv[n'])
    batch = np.asarray(batch, np.int64)[inv]
    bcol = np.full((NC, 128, NT), 999.0, np.float16)
    for c in range(NC):
        bc_ = batch[c * NSH:(c + 1) * NSH].astype(np.float16)
        pad = np.full(NPAD - NSH, 999.0, np.float16)
        bcol[c] = np.concatenate([bc_, pad]).reshape(NT, 128).T
    cnt_g = np.bincount(batch, minlength=G).astype(f32)
    cntrecip = (1.0 / np.maximum(cnt_g, 1.0)).reshape(G, 1)

    # folded weights
    W1 = np.asarray(W1, f32); W2 = np.asarray(W2, f32)
    s1 = np.asarray(bn1_g, f32) / np.sqrt(np.asarray(bn1_v, f32) + BN_EPS)
    t1 = (np.asarray(bias1, f32) - np.asarray(bn1_m, f32)) * s1 + np.asarray(bn1_b, f32)
    s2 = np.asarray(bn2_g, f32) / np.sqrt(np.asarray(bn2_v, f32) + BN_EPS)
    t2 = (np.asarray(bias2, f32) - np.asarray(bn2_m, f32)) * s2 + np.asarray(bn2_b, f32)
    aS1 = np.asarray(att_src1, f32)   # [H, HID]
    aD1 = np.asarray(att_dst1, f32)
    # per-head rotation Q (first column = att_src) and inverse R = Q^-1 * s1
    rng = np.random.default_rng(12345)
    Qblk = np.zeros((H * HID, H * HID), f32)
    Rpack = np.zeros((128, 2, 128), f32)     # blockdiag pairs of R_h
    for h in range(H):
        a = aS1[h]
        M = np.concatenate([a[:, None],
                            rng.standard_normal((HID, HID - 1)).astype(f32)], 1)
        Qf, _ = np.linalg.qr(M)
        Q = np.concatenate([a[:, None], Qf[:, 1:]], 1).astype(f32)
        R = np.linalg.inv(Q).astype(f32) * s1[h * HID:(h + 1) * HID][None, :]
        Qblk[h * HID:(h + 1) * HID, h * HID:(h + 1) * HID] = Q
        b2, r2_ = divmod(h, 2)
        Rpack[r2_ * HID:(r2_ + 1) * HID, b2, r2_ * HID:(r2_ + 1) * HID] = R
    AblkD = np.zeros((H * HID, H), f32)
    for h in range(H):
        AblkD[h * HID:(h + 1) * HID, h] = aD1[h]
    W1e = np.concatenate([W1 @ Qblk, W1 @ AblkD], axis=1)            # [128, 260]
    t1colT = t1.reshape(2, 128).T.copy()                             # [128, 2]
    aS2 = np.asarray(att_src2, f32).reshape(HID)
    aD2 = np.asarray(att_dst2, f32).reshape(HID)
    W2e = np.concatenate([W2 * s2[None, :], (W2 @ aS2)[:, None],
                          (W2 @ aD2)[:, None]], axis=1)              # [256, 66]
    t2row = np.concatenate([t2, np.zeros(2, f32)]).reshape(1, 66)

    iotam = np.tile(np.arange(128, dtype=np.float16), (128, 1))
    iota64 = np.tile(np.arange(64, dtype=np.float16), (128, 1))
    ident = np.eye(128, dtype=f32)
    identh = np.eye(128, dtype=np.float16)
    onesrow = np.ones((1, 128), f32)

    x = np.asarray(x, f32)[inv]
    in_maps = []
    for c in range(NC):
        xs = x[c * NSH:(c + 1) * NSH]
        xT = np.zeros((DIN, NPAD), np.float16)
        xT[:, :NSH] = xs.T.astype(np.float16)
        in_maps.append({
            "xT": xT, "gidx": gidx[c], "dcol": dcol[c],
            "bcol": bcol[c].copy(),
            "W1e": W1e.astype(np.float16), "t1colT": t1colT,
            "Rpack": Rpack.astype(np.float16),
            "W2e": W2e.reshape(2, 128, 66).transpose(1, 0, 2).astype(np.float16).copy(),
            "t2row": t2row,
            "iotam": iotam, "iota64": iota64, "ident": ident, "identh": identh,
            "onesrow": onesrow,
            "Wc1": np.asarray(Wc1, f32), "bc1row": np.asarray(bc1, f32).reshape(1, HID),
            "Wc2": np.asarray(Wc2, f32), "bc2row": np.asarray(bc2, f32).reshape(1, 2),
            "cntrecip": cntrecip,
        })
    return in_maps, CH, choff, CHUNKS


def _build(CH, choff, CHUNKS):
    AluOp = mybir.AluOpType
    Act = mybir.ActivationFunctionType
    nc = bacc.Bacc("TRN2", target_bir_lowering=False, debug=False, num_devices=NC,
                   num_swdge_queues=4)

    xT_d = nc.dram_tensor("xT", [DIN, NPAD], dt.float16, kind="ExternalInput")
    gidx_d = nc.dram_tensor("gidx", [128, CHUNKS * 8], dt.int16, kind="ExternalInput")
    dcol_d = nc.dram_tensor("dcol", [128, CHUNKS], dt.float16, kind="ExternalInput")
    bcol_d = nc.dram_tensor("bcol", [128, NT], dt.float16, kind="ExternalInput")
    W1e_d = nc.dram_tensor("W1e", [DIN, 260], dt.float16, kind="ExternalInput")
    t1colT_d = nc.dram_tensor("t1colT", [128, 2], dt.float32, kind="ExternalInput")
    Rpack_d = nc.dram_tensor("Rpack", [128, 2, 128], dt.float16, kind="ExternalInput")
    W2e_d = nc.dram_tensor("W2e", [128, 2, 66], dt.float16, kind="ExternalInput")
    t2row_d = nc.dram_tensor("t2row", [1, 66], dt.float32, kind="ExternalInput")
    iotam_d = nc.dram_tensor("iotam", [128, 128], dt.float16, kind="ExternalInput")
    iota64_d = nc.dram_tensor("iota64", [128, 64], dt.float16, kind="ExternalInput")
    ident_d = nc.dram_tensor("ident", [128, 128], dt.float32, kind="ExternalInput")
    identh_d = nc.dram_tensor("identh", [128, 128], dt.float16, kind="ExternalInput")
    ones_d = nc.dram_tensor("onesrow", [1, 128], dt.float32, kind="ExternalInput")
    Wc1_d = nc.dram_tensor("Wc1", [HID, HID], dt.float32, kind="ExternalInput")
    bc1_d = nc.dram_tensor("bc1row", [1, HID], dt.float32, kind="ExternalInput")
    Wc2_d = nc.dram_tensor("Wc2", [HID, 2], dt.float32, kind="ExternalInput")
    bc2_d = nc.dram_tensor("bc2row", [1, 2], dt.float32, kind="ExternalInput")
    crec_d = nc.dram_tensor("cntrecip", [G, 1], dt.float32, kind="ExternalInput")
    out_d = nc.dram_tensor("out", [G, 2], dt.float32, kind="ExternalOutput")

    RG = [list(range(NC))]
    AGC = 2                      # allgather chunks per table
    bounds = [0] + [((i + 1) * NT // AGC) * 128 for i in range(AGC)]

    with tile.TileContext(nc) as tc:
        with (
            tc.tile_pool(name="const", bufs=1) as cp,
            tc.tile_pool(name="sb", bufs=3) as sb,
            tc.tile_pool(name="gbuf", bufs=3) as gp,
            tc.tile_pool(name="gbuf2", bufs=2) as gp2,
            tc.tile_pool(name="small", bufs=4) as sp,
            tc.tile_pool(name="ps", bufs=2, space="PSUM") as ps,
            tc.tile_pool(name="pspool", bufs=1, space="PSUM") as psp,
            tc.tile_pool(name="dram", bufs=1, space="DRAM") as dram,
        ):
            nc.gpsimd.load_library(library_config.mlp)

            # ---- consts to SBUF
            def cload(dten, shape, dtype):
                tl = cp.tile(shape, dtype, tag=dten.name)
                nc.sync.dma_start(tl[:], dten[:])
                return tl
            W1e = cload(W1e_d, [DIN, 260], dt.float16)
            t1colT = cload(t1colT_d, [128, 2], dt.float32)
            Rpack = cload(Rpack_d, [128, 2, 128], dt.float16)
            W2e = cload(W2e_d, [128, 2, 66], dt.float16)
            t2row = cload(t2row_d, [1, 66], dt.float32)
            iotam = cload(iotam_d, [128, 128], dt.float16)
            iota64 = cload(iota64_d, [128, 64], dt.float16)
            ident = cload(ident_d, [128, 128], dt.float32)
            identh = cload(identh_d, [128, 128], dt.float16)
            ones = cload(ones_d, [1, 128], dt.float32)
            Wc1 = cload(Wc1_d, [HID, HID], dt.float32)
            bc1row = cload(bc1_d, [1, HID], dt.float32)
            Wc2 = cload(Wc2_d, [HID, 2], dt.float32)
            bc2row = cload(bc2_d, [1, 2], dt.float32)
            cntrecip = cload(crec_d, [G, 1], dt.float32)
            gidx = cload(gidx_d, [128, CHUNKS * 8], dt.int16)
            dcol = cload(dcol_d, [128, CHUNKS], dt.float16)
            bcol = cload(bcol_d, [128, NT], dt.float16)

            ad1 = cp.tile([128, NT, H], dt.float32, tag="ad1")
            ad2 = cp.tile([128, NT, 1], dt.float32, tag="ad2")
            poh = cp.tile([128, NT, G], dt.float16, tag="poh")
            # pooling one-hot (built once)
            nc.vector.tensor_tensor(
                poh[:],
                iota64[:].unsqueeze(1).broadcast_to([128, NT, G]),
                bcol[:].unsqueeze(2).broadcast_to([128, NT, G]),
                AluOp.is_equal)

            # ---- DRAM tables (collective outputs in Shared space)
            t1stage = dram.tile([NPAD, RF1], dt.float16)
            t1full = nc.dram_tensor("t1full", [NC * NPAD, RF1], dt.float16,
                                    kind="Internal", addr_space="Shared").ap()
            t2stage = dram.tile([NPAD, RF2], dt.float16)
            t2full = nc.dram_tensor("t2full", [NC * NPAD, RF2], dt.float16,
                                    kind="Internal", addr_space="Shared").ap()

            def ag_chunk(stage, full, ci):
                r0, r1 = bounds[ci], bounds[ci + 1]
                o0, o1 = NC * r0, NC * r1
                nc.gpsimd.collective_compute(
                    "AllGather", mybir.AluOpType.bypass, replica_groups=RG,
                    ins=[stage[r0:r1, :].opt()],
                    outs=[full[o0:o1, :].opt()])

            # ================= PHASE A: L1 head (y table + alpha_dst) =====
            with nc.named_scope("phaseA"), tc.tile_pool(name="head", bufs=3) as hp:
                ci = 0
                for t in range(NT):
                    xTt = hp.tile([DIN, 128], dt.float16, tag="xTt")
                    nc.sync.dma_start(xTt[:], xT_d[:, t * 128:(t + 1) * 128])
                    pa = ps.tile([128, 512], dt.float32, tag="pep")
                    nc.tensor.matmul(pa[:, 0:260], xTt[:], W1e[:],
                                     start=True, stop=True)
                    tab = sb.tile([128, RF1], dt.float16, tag="tab1")
                    nc.scalar.activation(tab[:], pa[:, 0:256], Act.Copy)
                    nc.vector.tensor_copy(ad1[:, t, :], pa[:, 256:260])
                    nc.sync.dma_start(t1stage[t * 128:(t + 1) * 128, :], tab[:])
                    if (t + 1) * 128 == bounds[ci + 1]:
                        ag_chunk(t1stage, t1full, ci)
                        ci += 1

            # ================= PHASE B: L1 edges + L2 head ================
            def edge_phase(layer, tfull, rfw, nh, adt, adrow_tag):
                """One GAT edge phase. Yields per-tile (t, hsb-or-h1T)."""
                halves = (tfull[0:HALF, :], tfull[HALF:2 * HALF, :])
                ncol = nh * HID   # message feature cols (256 / 64)
                rot = (layer == 1)
                maxc = int((CH[:, 0] + CH[:, 1]).max())
                for t in range(NT):
                    ct0 = int(choff[t, 0]); n0 = int(CH[t, 0])
                    ct1 = int(choff[t, 1]); n1 = int(CH[t, 1])
                    ctot = n0 + n1
                    # adh | ones column (colsum distinguishes pad edges)
                    adh = sp.tile([128, nh + 1], dt.float16, tag=adrow_tag + "h")
                    nc.scalar.activation(adh[:, 0:nh], adt[:, t, :], Act.Copy)
                    nc.vector.memset(adh[:, nh:nh + 1], 1.0)

                    gb = gp.tile([128, int(CH[:, 0].max() + CH[:, 1].max()), rfw],
                                 dt.float16, tag=f"gb{layer}")
                    for gi, (hoff, nch) in enumerate(((ct0, n0), (ct1, n1))):
                        boff = 0 if gi == 0 else n0
                        nc.gpsimd.dma_gather(
                            gb[:, boff:boff + nch, :], halves[gi],
                            gidx[:, hoff * 8:(hoff + nch) * 8],
                            num_idxs=nch * 128, num_idxs_reg=nch * 128,
                            elem_size=rfw, queue_num=(t * 2 + gi) % 4,
                            single_packet=(nch * 128 <= 1024))
                    # one-hot for all chunks of this tile
                    oh = gp2.tile([128, maxc, 128], dt.float16, tag="oh")
                    dc = dcol[:, ct0:ct0 + ctot]  # groups contiguous per tile
                    nc.vector.tensor_tensor(
                        oh[:, 0:ctot, :],
                        iotam[:].unsqueeze(1).broadcast_to([128, ctot, 128]),
                        dc.unsqueeze(2).broadcast_to([128, ctot, 128]),
                        AluOp.is_equal)
                    # one-hot transpose on the PE (8 chunks per psum bank),
                    # copied to SBUF for the alpha_dst matmuls
                    ohT = gp2.tile([128, maxc * 128], dt.float16, tag="ohT")
                    for b0 in range(0, ctot, 8):
                        bn = min(8, ctot - b0)
                        ott = ps.tile([128, 512], dt.float32, tag="ptt")
                        oth = ott[:].bitcast(dt.float16)
                        for j in range(bn):
                            nc.tensor.transpose(oth[:, j * 128:(j + 1) * 128],
                                                oh[:, b0 + j, :], identh[:])
                        nc.scalar.copy(ohT[:, b0 * 128:(b0 + bn) * 128],
                                       oth[:, 0:bn * 128])
                    # [alpha_dst | colsum] per edge: ohT.T @ adh per chunk
                    ade = psp.tile([128, maxc, nh + 1], dt.float32, tag="pexp")
                    for c in range(ctot):
                        nc.tensor.matmul(
                            ade[:, c, :],
                            ohT[:, c * 128:(c + 1) * 128], adh[:],
                            start=True, stop=True)
                    # e = a_src + a_dst - 25*(1 - colsum); pads end at ~-13
                    # so exp() stays tiny even on stale pad data
                    asb = sp.tile([128, maxc, nh], dt.float32, tag=f"as{layer}")
                    if rot:
                        as_ap = (gb[:, 0:ctot, :]
                                 .rearrange("p c (h f) -> p c h f", h=nh)
                                 [:, :, :, 0:1].squeeze(3))
                    else:
                        as_ap = gb[:, 0:ctot, ncol:ncol + 2 * nh].bitcast(dt.float32)
                    nc.vector.tensor_scalar(asb[:, 0:ctot, :], as_ap,
                                            12.0, None, AluOp.min)
                    ee = sp.tile([128, maxc, nh], dt.float32, tag=f"ee{layer}")
                    nc.vector.scalar_tensor_tensor(
                        ee[:, 0:ctot, :],
                        ade[:, 0:ctot, nh:nh + 1]
                        .broadcast_to([128, ctot, nh]),
                        25.0, asb[:, 0:ctot, :], AluOp.mult, AluOp.add)
                    nc.vector.scalar_tensor_tensor(
                        ee[:, 0:ctot, :], ee[:, 0:ctot, :],
                        25.0, ade[:, 0:ctot, 0:nh], AluOp.subtract, AluOp.add)
                    nc.vector.scalar_tensor_tensor(
                        ee[:, 0:ctot, :], ee[:, 0:ctot, :], NEG_SLOPE,
                        ee[:, 0:ctot, :], AluOp.mult, AluOp.max)
                    ex = sp.tile([128, maxc, nh], dt.float16, tag=f"ex{layer}")
                    nc.scalar.activation(ex[:, 0:ctot, :], ee[:, 0:ctot, :],
                                         Act.Exp)
                    # scale messages by exp (in place, per head block)
                    nc.vector.tensor_tensor(
                        gb[:, 0:ctot, 0:ncol].rearrange(
                            "p c (h f) -> p c h f", h=nh),
                        gb[:, 0:ctot, 0:ncol].rearrange(
                            "p c (h f) -> p c h f", h=nh),
                        ex[:, 0:ctot, :].unsqueeze(3)
                          .broadcast_to([128, ctot, nh, HID]),
                        AluOp.mult)
                    # aggregate: messages and exp-sums into one psum bank
                    pb = ps.tile([128, ncol + nh], dt.float32, tag="pagg")
                    for c in range(ctot):
                        nc.tensor.matmul(pb[:, 0:ncol], oh[:, c, :],
                                         gb[:, c, 0:ncol],
                                         start=(c == 0), stop=(c == ctot - 1))
                        nc.tensor.matmul(pb[:, ncol:ncol + nh], oh[:, c, :],
                                         ex[:, c, :],
                                         start=(c == 0), stop=(c == ctot - 1))
                    # epilogue: h = num / (den + eps), relu (+ unrotation L1)
                    den = sp.tile([128, nh], dt.float32, tag=f"den{layer}")
                    nc.vector.tensor_scalar(den[:], pb[:, ncol:ncol + nh],
                                            1e-16, None, AluOp.add)
                    rec = sp.tile([128, nh], dt.float32, tag=f"rec{layer}")
                    nc.vector.reciprocal(rec[:], den[:])
                    if not rot:
                        hsb = sb.tile([128, ncol], dt.float16, tag=f"h{layer}")
                        nc.scalar.activation(hsb[:], pb[:, 0:ncol], Act.Relu,
                                             scale=rec[:, 0:1])
                        yield t, hsb
                        continue
                    hy = sb.tile([128, ncol], dt.float32, tag="hy")
                    for h in range(nh):
                        nc.scalar.activation(
                            hy[:, h * HID:(h + 1) * HID],
                            pb[:, h * HID:(h + 1) * HID], Act.Copy,
                            scale=rec[:, h:h + 1])
                    # transpose agg_y, unrotate (R = Q^-1 s1), bias+relu -> h1T
                    yT = sb.tile([128, 2, 128], dt.float16, tag="yT")
                    for k in range(2):
                        pt = ps.tile([128, 512], dt.float32, tag="pep")
                        nc.tensor.transpose(pt[:, 0:128],
                                            hy[:, k * 128:(k + 1) * 128],
                                            ident[:])
                        nc.scalar.copy(yT[:, k, :], pt[:, 0:128])
                    h1T = sb.tile([128, 2, 128], dt.float16, tag="h1T")
                    for k in range(2):
                        p2 = ps.tile([128, 512], dt.float32, tag="pep")
                        nc.tensor.matmul(p2[:, 0:128], Rpack[:, k, :],
                                         yT[:, k, :],
                                         start=True, stop=True)
                        nc.scalar.activation(h1T[:, k, :], p2[:, 0:128],
                                             Act.Relu, bias=t1colT[:, k:k + 1])
                    yield t, h1T

            # L1 edge phase; fused L2 head per tile
            ci = 0
            for t, h1T in edge_phase(1, t1full, RF1, H, ad1, "a1"):
                pc = ps.tile([128, 512], dt.float32, tag="pep")
                for k in range(2):
                    nc.tensor.matmul(pc[:, 0:66], h1T[:, k, :], W2e[:, k, :],
                                     start=(k == 0), stop=False)
                nc.tensor.matmul(pc[:, 0:66], ones[0:1, :], t2row[:],
                                 start=False, stop=True)
                tab2 = sb.tile([128, RF2], dt.float16, tag="tab2")
                nc.scalar.activation(tab2[:, 0:HID], pc[:, 0:HID], Act.Copy)
                nc.vector.tensor_copy(tab2[:, HID:HID + 2].bitcast(dt.float32),
                                      pc[:, HID:HID + 1])
                nc.vector.tensor_copy(ad2[:, t, :], pc[:, HID + 1:HID + 2])
                nc.vector.memset(tab2[:, HID + 2:RF2], 0.0)
                nc.sync.dma_start(t2stage[t * 128:(t + 1) * 128, :], tab2[:])
                if (t + 1) * 128 == bounds[ci + 1]:
                    ag_chunk(t2stage, t2full, ci)
                    ci += 1

            # ================= PHASE C: L2 edges + pooling ================
            pgsum = psp.tile([G, HID], dt.float32, tag="pgsum")
            for t, h2 in edge_phase(2, t2full, RF2, 1, ad2, "a2"):
                nc.tensor.matmul(pgsum[:], poh[:, t, :], h2[:],
                                 start=(t == 0), stop=(t == NT - 1))

            # ================= PHASE D: AllReduce + classifier ============
            ar_in = dram.tile([G, HID], dt.float32)
            ar_out = nc.dram_tensor("ar_out", [G, HID], dt.float32,
                                    kind="Internal", addr_space="Shared").ap()
            psum_sb = sb.tile([G, HID], dt.float32, tag="psum_sb")
            nc.vector.tensor_copy(psum_sb[:], pgsum[:])
            nc.sync.dma_start(ar_in[:], psum_sb[:])
            nc.gpsimd.collective_compute(
                "AllReduce", mybir.AluOpType.add, replica_groups=RG,
                ins=[ar_in.opt()], outs=[ar_out.opt()])
            rep = sb.tile([G, HID], dt.float32, tag="rep")
            nc.sync.dma_start(rep[:], ar_out[:])
            nc.vector.tensor_scalar(rep[:], rep[:], cntrecip[:, 0:1], None,
                                    AluOp.mult)
            # hc = relu(rep @ Wc1 + bc1)
            ptr = ps.tile([128, 512], dt.float32, tag="pep")
            nc.tensor.transpose(ptr[0:G, 0:G], rep[:], ident[0:G, 0:G])
            repT = sb.tile([G, G], dt.float32, tag="repT")
            nc.scalar.copy(repT[:], ptr[0:G, 0:G])
            ph = ps.tile([128, 512], dt.float32, tag="pep")
            nc.tensor.matmul(ph[0:G, 0:HID], repT[:], Wc1[:], start=True, stop=False)
            nc.tensor.matmul(ph[0:G, 0:HID], ones[0:1, 0:G], bc1row[:],
                             start=False, stop=True)
            hc = sb.tile([G, HID], dt.float32, tag="hc")
            nc.scalar.activation(hc[:], ph[0:G, 0:HID], Act.Relu)
            pt2 = ps.tile([128, 512], dt.float32, tag="pep")
            nc.tensor.transpose(pt2[0:G, 0:G], hc[:], ident[0:G, 0:G])
            hcT = sb.tile([G, G], dt.float32, tag="hcT")
            nc.scalar.copy(hcT[:], pt2[0:G, 0:G])
            pl = ps.tile([128, 512], dt.float32, tag="pep")
            nc.tensor.matmul(pl[0:G, 0:2], hcT[:], Wc2[:], start=True, stop=False)
            nc.tensor.matmul(pl[0:G, 0:2], ones[0:1, 0:G], bc2row[:],
                             start=False, stop=True)
            # log softmax over the 2 logits
            lg = sb.tile([G, 2], dt.float32, tag="lg")
            nc.vector.tensor_copy(lg[:], pl[0:G, 0:2])
            mx = sb.tile([G, 1], dt.float32, tag="mx")
            nc.vector.tensor_reduce(mx[:], lg[:], mybir.AxisListType.X,
                                    AluOp.max)
            nc.vector.tensor_scalar(lg[:], lg[:], mx[:, 0:1], None,
                                    AluOp.subtract)
            ex = sb.tile([G, 2], dt.float32, tag="ex")
            nc.scalar.activation(ex[:], lg[:], Act.Exp)
            sm = sb.tile([G, 1], dt.float32, tag="sm")
            nc.vector.tensor_reduce(sm[:], ex[:], mybir.AxisListType.X,
                                    AluOp.add)
            ls = sb.tile([G, 1], dt.float32, tag="ls")
            nc.scalar.activation(ls[:], sm[:], Act.Ln)
            outv = sb.tile([G, 2], dt.float32, tag="outv")
            nc.vector.tensor_scalar(outv[:], lg[:], ls[:, 0:1], None,
                                    AluOp.subtract)
            nc.sync.dma_start(out_d[:], outv[:])

    nc.compile()
    return nc


_last_result = [None]


def kernel(**inputs):
    import hashlib
    configure(int(np.asarray(inputs["x"]).shape[0]),
              int(np.asarray(inputs["edge_index"]).shape[1]))
    ek = np.ascontiguousarray(np.asarray(inputs["edge_index"]))
    bk = np.ascontiguousarray(np.asarray(inputs["batch"]))
    key = hashlib.sha1(ek.tobytes() + bk.tobytes()).hexdigest()
    in_maps, CH, choff, CHUNKS = _prep_host(**inputs)
    if key not in _cache:
        _cache[key] = _build(CH, choff, CHUNKS)
    nc = _cache[key]
    res = bass_utils.run_bass_kernel_spmd(nc, in_maps, core_ids=list(range(NC)))
    _last_result[0] = res
    return res.results[0]["out"].astype(np.float32)


def kernel_exec_ns():
    r = _last_result[0]
    return None if r is None else r.exec_time_ns


# revision 30
# speedup vs baseline: 1.5672x; 1.5672x over previous
"""GAT (2-layer, 4->1 heads) + global mean pool + classifier on 8 trn2 NeuronCores.

Sharding: nodes (and their incoming edges) partitioned contiguously across the
8 cores; small weights replicated; per-layer node-feature tables AllGathered
(chunked, overlapped with producer compute); per-graph pooled sums AllReduced.

Layer-1 table rows are rotated per head (first basis vector = att_src) so the
512B row carries both the message features and alpha_src; the inverse rotation
is folded into the transposed epilogue feeding the layer-2 head. The edge
one-hot transpose (for the alpha_dst lookup) is done on the PE instead of a
broadcast DMA + compare.

Self-contained: takes full inputs, returns full [64, 2] log-softmax output.
"""
import sys
for _p in ('/opt/trn_rl_repo', '/root/.axon_site/_ro/trn_rl_repo'):
    if _p not in sys.path:
        sys.path.insert(0, _p)

import numpy as np
import concourse.bass as bass
import concourse.bacc as bacc
import concourse.tile as tile
import concourse.mybir as mybir
from concourse import bass_utils, library_config

dt = mybir.dt

# problem constants (hardcoded per contract); N/E re-derivable for sim tests
N = 50000
E = 1600000
G = 64
DIN = 128
HID = 64
H = 4
NEG_SLOPE = 0.2
BN_EPS = 1e-5
NC = 8
RF1 = 256                # L1 table row: fp16 slots (512B): y = xw @ Q (as = y[h,0])
RF2 = 128                # L2 table row: fp16 slots (256B): 64 xws | 2 a_src(f32) | pad


def configure(n, e):
    global N, E, NSH, NT, NPAD, HALF
    N, E = n, e
    NSH = N // NC
    NT = (NSH + 127) // 128
    NPAD = NT * 128
    HALF = NC // 2 * NPAD


configure(N, E)

_cache = {}


def _prep_host(x, edge_index, batch,
               W1, att_src1, att_dst1, bias1, bn1_g, bn1_b, bn1_m, bn1_v,
               W2, att_src2, att_dst2, bias2, bn2_g, bn2_b, bn2_m, bn2_v,
               Wc1, bc1, Wc2, bc2):
    """Index-space layout + folded weights. Returns (in_maps, CH, meta)."""
    f32 = np.float32
    src = np.concatenate([np.asarray(edge_index[0], np.int64),
                          np.arange(N, dtype=np.int64)])
    dst = np.concatenate([np.asarray(edge_index[1], np.int64),
                          np.arange(N, dtype=np.int64)])
    EE = src.shape[0]

    # relabel nodes: snake-assign by in-degree so every (core, tile) bucket
    # has a near-equal edge count (balances chunk counts across cores)
    indeg = np.bincount(dst, minlength=N)
    order_by_deg = np.argsort(-indeg, kind='stable')
    nbuckets = NC * NT
    newid = np.empty(N, np.int64)
    bidx = np.arange(N) % nbuckets
    snake = (np.arange(N) // nbuckets) % 2 == 1
    bidx = np.where(snake, nbuckets - 1 - bidx, bidx)
    # bucket b corresponds to core b // NT, tile b % NT
    slot_in_bucket = np.zeros(N, np.int64)
    counts = np.zeros(nbuckets, np.int64)
    for i in range(N):
        b = bidx[i]
        while counts[b] >= 128:
            b = (b + 1) % nbuckets
        slot_in_bucket[i] = counts[b]
        counts[b] += 1
        bidx[i] = b
    cores_of = bidx // NT
    tiles_of = bidx % NT
    newid[order_by_deg] = cores_of * NSH + tiles_of * 128 + slot_in_bucket
    # tile NT-1 slots beyond NSH are ghosts; ensure none assigned
    lastcap = NSH - (NT - 1) * 128
    bad = (tiles_of == NT - 1) & (slot_in_bucket >= lastcap)
    if bad.any():
        ov = np.where(bad)[0]
        free_buckets = [b for b in range(nbuckets)
                        if (b % NT != NT - 1 and counts[b] < 128)
                        or (b % NT == NT - 1 and counts[b] < lastcap)]
        fi = 0
        for i in ov:
            while True:
                b = free_buckets[fi % len(free_buckets)]
                cap = 128 if b % NT != NT - 1 else lastcap
                if counts[b] < cap:
                    break
                fi += 1
            slot_in_bucket[i] = counts[b]
            counts[b] += 1
            cores_of[i] = b // NT
            tiles_of[i] = b % NT
            fi += 1
        newid[order_by_deg] = cores_of * NSH + tiles_of * 128 + slot_in_bucket
    inv = np.empty(N, np.int64)
    inv[newid] = np.arange(N)      # inv[new] = old
    src = newid[src]
    dst = newid[dst]

    core = dst // NSH
    ldst = dst - core * NSH
    t = ldst >> 7                     # dst tile within shard
    dit = ldst & 127                  # dst index within tile
    # table rows laid out chunk-major (per allgather chunk, core-major
    # inside) so each chunk's collective output is contiguous
    tA = NT // 2
    rowsA = tA * 128
    rowsB = NPAD - rowsA
    ls = src % NSH
    scA = ls < rowsA
    trow = np.where(scA, (src // NSH) * rowsA + ls,
                    NC * rowsA + (src // NSH) * rowsB + (ls - rowsA))
    g = (trow >= HALF).astype(np.int64)           # table half by row
    lidx = trow - g * HALF                        # int16-safe local row

    key = ((core * NT + t) * 2 + g)   # bucket id, core-major
    nbuck = NC * NT * 2
    cnt = np.bincount(key, minlength=nbuck).reshape(NC, NT, 2)
    CH = np.maximum(1, (cnt.max(axis=0) + 127) // 128)   # [NT, 2] shared chunks
    CH = CH + (CH & 1)   # even counts -> 4B-aligned stream slice offsets
    CHUNKS = int(CH.sum())
    choff = np.zeros((NT, 2), np.int64)
    choff.reshape(-1)[1:] = np.cumsum(CH.reshape(-1))[:-1]

    # stable-sort edges by bucket; ranks within bucket
    order = np.argsort(key, kind='stable')
    skey = key[order]
    bstart = np.searchsorted(skey, np.arange(nbuck))
    rank = np.arange(EE, dtype=np.int64) - bstart[skey]
    # padded stream position (per core stream of CHUNKS*128 slots)
    bt = (skey // 2) % NT
    bg = skey % 2
    pos = choff[bt, bg] * 128 + rank
    scor = skey // (NT * 2)

    gidx_all = np.zeros((NC, CHUNKS * 128), np.int16)
    dcol_all = np.full((NC, CHUNKS * 128), 999.0, np.float16)
    for c in range(NC):
        m = scor == c
        eidx = order[m]
        gidx_all[c, pos[m]] = lidx[eidx].astype(np.int16)
        dcol_all[c, pos[m]] = dit[eidx].astype(np.float16)

    # wrap: element i -> [i % 16, i // 16] / dstcol: chunk-major -> [128, CHUNKS]
    gidx = np.tile(gidx_all.reshape(NC, CHUNKS * 8, 16).transpose(0, 2, 1), (1, 8, 1)).copy()
    dcol = dcol_all.reshape(NC, CHUNKS, 128).transpose(0, 2, 1).copy()

    # batch / pooling (note: node n' holds old node inv[n'])
    batch = np.asarray(batch, np.int64)[inv]
    bcol = np.full((NC, 128, NT), 999.0, np.float16)
    for c in range(NC):
        bc_ = batch[c * NSH:(c + 1) * NSH].astype(np.float16)
        pad = np.full(NPAD - NSH, 999.0, np.float16)
        bcol[c] = np.concatenate([bc_, pad]).reshape(NT, 128).T
    cnt_g = np.bincount(batch, minlength=G).astype(f32)
    cntrecip = (1.0 / np.maximum(cnt_g, 1.0)).reshape(G, 1)

    # folded weights
    W1 = np.asarray(W1, f32); W2 = np.asarray(W2, f32)
    s1 = np.asarray(bn1_g, f32) / np.sqrt(np.asarray(bn1_v, f32) + BN_EPS)
    t1 = (np.asarray(bias1, f32) - np.asarray(bn1_m, f32)) * s1 + np.asarray(bn1_b, f32)
    s2 = np.asarray(bn2_g, f32) / np.sqrt(np.asarray(bn2_v, f32) + BN_EPS)
    t2 = (np.asarray(bias2, f32) - np.asarray(bn2_m, f32)) * s2 + np.asarray(bn2_b, f32)
    aS1 = np.asarray(att_src1, f32)   # [H, HID]
    aD1 = np.asarray(att_dst1, f32)
    # per-head rotation Q (first column = att_src) and inverse R = Q^-1 * s1
    rng = np.random.default_rng(12345)
    Qblk = np.zeros((H * HID, H * HID), f32)
    Rpack = np.zeros((128, 2, 128), f32)     # blockdiag pairs of R_h
    for h in range(H):
        a = aS1[h]
        M = np.concatenate([a[:, None],
                            rng.standard_normal((HID, HID - 1)).astype(f32)], 1)
        Qf, _ = np.linalg.qr(M)
        Q = np.concatenate([a[:, None], Qf[:, 1:]], 1).astype(f32)
        R = np.linalg.inv(Q).astype(f32) * s1[h * HID:(h + 1) * HID][None, :]
        Qblk[h * HID:(h + 1) * HID, h * HID:(h + 1) * HID] = Q
        b2, r2_ = divmod(h, 2)
        Rpack[r2_ * HID:(r2_ + 1) * HID, b2, r2_ * HID:(r2_ + 1) * HID] = R
    AblkD = np.zeros((H * HID, H), f32)
    for h in range(H):
        AblkD[h * HID:(h + 1) * HID, h] = aD1[h]
    W1e = np.concatenate([W1 @ Qblk, W1 @ AblkD], axis=1)            # [128, 260]
    t1colT = t1.reshape(2, 128).T.copy()                             # [128, 2]
    aS2 = np.asarray(att_src2, f32).reshape(HID)
    aD2 = np.asarray(att_dst2, f32).reshape(HID)
    W2e = np.concatenate([W2 * s2[None, :], (W2 @ aS2)[:, None],
                          (W2 @ aD2)[:, None]], axis=1)              # [256, 66]
    t2row = np.concatenate([t2, np.zeros(2, f32)]).reshape(1, 66)

    iotam = np.tile(np.arange(128, dtype=np.float16), (128, 1))
    iota64 = np.tile(np.arange(64, dtype=np.float16), (128, 1))
    ident = np.eye(128, dtype=f32)
    identh = np.eye(128, dtype=np.float16)
    onesrow = np.ones((1, 128), f32)

    x = np.asarray(x, f32)[inv]
    in_maps = []
    for c in range(NC):
        xs = x[c * NSH:(c + 1) * NSH]
        xT = np.zeros((DIN, NPAD), np.float16)
        xT[:, :NSH] = xs.T.astype(np.float16)
        in_maps.append({
            "xT": xT, "gidx": gidx[c], "dcol": dcol[c],
            "bcol": bcol[c].copy(),
            "W1e": W1e.astype(np.float16), "t1colT": t1colT,
            "Rpack": Rpack.astype(np.float16),
            "W2e": W2e.reshape(2, 128, 66).transpose(1, 0, 2).astype(np.float16).copy(),
            "t2row": t2row,
            "iotam": iotam, "iota64": iota64, "ident": ident, "identh": identh,
            "onesrow": onesrow,
            "Wc1": np.asarray(Wc1, f32), "bc1row": np.asarray(bc1, f32).reshape(1, HID),
            "Wc2": np.asarray(Wc2, f32), "bc2row": np.asarray(bc2, f32).reshape(1, 2),
            "cntrecip": cntrecip,
        })
    return in_maps, CH, choff, CHUNKS


def _build(CH, choff, CHUNKS):
    AluOp = mybir.AluOpType
    Act = mybir.ActivationFunctionType
    nc = bacc.Bacc("TRN2", target_bir_lowering=False, debug=False, num_devices=NC,
                   num_swdge_queues=4)

    xT_d = nc.dram_tensor("xT", [DIN, NPAD], dt.float16, kind="ExternalInput")
    gidx_d = nc.dram_tensor("gidx", [128, CHUNKS * 8], dt.int16, kind="ExternalInput")
    dcol_d = nc.dram_tensor("dcol", [128, CHUNKS], dt.float16, kind="ExternalInput")
    bcol_d = nc.dram_tensor("bcol", [128, NT], dt.float16, kind="ExternalInput")
    W1e_d = nc.dram_tensor("W1e", [DIN, 260], dt.float16, kind="ExternalInput")
    t1colT_d = nc.dram_tensor("t1colT", [128, 2], dt.float32, kind="ExternalInput")
    Rpack_d = nc.dram_tensor("Rpack", [128, 2, 128], dt.float16, kind="ExternalInput")
    W2e_d = nc.dram_tensor("W2e", [128, 2, 66], dt.float16, kind="ExternalInput")
    t2row_d = nc.dram_tensor("t2row", [1, 66], dt.float32, kind="ExternalInput")
    iotam_d = nc.dram_tensor("iotam", [128, 128], dt.float16, kind="ExternalInput")
    iota64_d = nc.dram_tensor("iota64", [128, 64], dt.float16, kind="ExternalInput")
    ident_d = nc.dram_tensor("ident", [128, 128], dt.float32, kind="ExternalInput")
    identh_d = nc.dram_tensor("identh", [128, 128], dt.float16, kind="ExternalInput")
    ones_d = nc.dram_tensor("onesrow", [1, 128], dt.float32, kind="ExternalInput")
    Wc1_d = nc.dram_tensor("Wc1", [HID, HID], dt.float32, kind="ExternalInput")
    bc1_d = nc.dram_tensor("bc1row", [1, HID], dt.float32, kind="ExternalInput")
    Wc2_d = nc.dram_tensor("Wc2", [HID, 2], dt.float32, kind="ExternalInput")
    bc2_d = nc.dram_tensor("bc2row", [1, 2], dt.float32, kind="ExternalInput")
    crec_d = nc.dram_tensor("cntrecip", [G, 1], dt.float32, kind="ExternalInput")
    out_d = nc.dram_tensor("out", [G, 2], dt.float32, kind="ExternalOutput")

    RG = [list(range(NC))]
    AGC = 2                      # allgather chunks per table
    bounds = [0] + [((i + 1) * NT // AGC) * 128 for i in range(AGC)]

    with tile.TileContext(nc) as tc:
        with (
            tc.tile_pool(name="const", bufs=1) as cp,
            tc.tile_pool(name="sb", bufs=3) as sb,
            tc.tile_pool(name="gbuf", bufs=3) as gp,
            tc.tile_pool(name="gbuf2", bufs=2) as gp2,
            tc.tile_pool(name="small", bufs=4) as sp,
            tc.tile_pool(name="ps", bufs=2, space="PSUM") as ps,
            tc.tile_pool(name="pspool", bufs=1, space="PSUM") as psp,
            tc.tile_pool(name="dram", bufs=1, space="DRAM") as dram,
        ):
            nc.gpsimd.load_library(library_config.mlp)

            # ---- consts to SBUF
            def cload(dten, shape, dtype):
                tl = cp.tile(shape, dtype, tag=dten.name)
                nc.sync.dma_start(tl[:], dten[:])
                return tl
            W1e = cload(W1e_d, [DIN, 260], dt.float16)
            t1colT = cload(t1colT_d, [128, 2], dt.float32)
            Rpack = cload(Rpack_d, [128, 2, 128], dt.float16)
            W2e = cload(W2e_d, [128, 2, 66], dt.float16)
            t2row = cload(t2row_d, [1, 66], dt.float32)
            iotam = cload(iotam_d, [128, 128], dt.float16)
            iota64 = cload(iota64_d, [128, 64], dt.float16)
            ident = cload(ident_d, [128, 128], dt.float32)
            identh = cload(identh_d, [128, 128], dt.float16)
            ones = cload(ones_d, [1, 128], dt.float32)
            Wc1 = cload(Wc1_d, [HID, HID], dt.float32)
            bc1row = cload(bc1_d, [1, HID], dt.float32)
            Wc2 = cload(Wc2_d, [HID, 2], dt.float32)
            bc2row = cload(bc2_d, [1, 2], dt.float32)
            cntrecip = cload(crec_d, [G, 1], dt.float32)
            gidx = cload(gidx_d, [128, CHUNKS * 8], dt.int16)
            dcol = cload(dcol_d, [128, CHUNKS], dt.float16)
            bcol = cload(bcol_d, [128, NT], dt.float16)

            ad1 = cp.tile([128, NT, H], dt.float32, tag="ad1")
            ad2 = cp.tile([128, NT, 1], dt.float32, tag="ad2")
            poh = cp.tile([128, NT, G], dt.float16, tag="poh")
            # pooling one-hot (built once)
            nc.vector.tensor_tensor(
                poh[:],
                iota64[:].unsqueeze(1).broadcast_to([128, NT, G]),
                bcol[:].unsqueeze(2).broadcast_to([128, NT, G]),
                AluOp.is_equal)

            # ---- DRAM tables (collective outputs in Shared space)
            t1stage = dram.tile([NPAD, RF1], dt.float16)
            t1full = nc.dram_tensor("t1full", [NC * NPAD, RF1], dt.float16,
                                    kind="Internal", addr_space="Shared").ap()
            t2stage = dram.tile([NPAD, RF2], dt.float16)
            t2full = nc.dram_tensor("t2full", [NC * NPAD, RF2], dt.float16,
                                    kind="Internal", addr_space="Shared").ap()

            def ag_chunk(stage, full, ci):
                r0, r1 = bounds[ci], bounds[ci + 1]
                o0, o1 = NC * r0, NC * r1
                nc.gpsimd.collective_compute(
                    "AllGather", mybir.AluOpType.bypass, replica_groups=RG,
                    ins=[stage[r0:r1, :].opt()],
                    outs=[full[o0:o1, :].opt()])

            # ================= PHASE A: L1 head (y table + alpha_dst) =====
            with nc.named_scope("phaseA"), tc.tile_pool(name="head", bufs=3) as hp:
                ci = 0
                for t in range(NT):
                    xTt = hp.tile([DIN, 128], dt.float16, tag="xTt")
                    nc.sync.dma_start(xTt[:], xT_d[:, t * 128:(t + 1) * 128])
                    pa = ps.tile([128, 512], dt.float32, tag="pep")
                    nc.tensor.matmul(pa[:, 0:260], xTt[:], W1e[:],
                                     start=True, stop=True)
                    tab = sb.tile([128, RF1], dt.float16, tag="tab1")
                    nc.scalar.activation(tab[:], pa[:, 0:256], Act.Copy)
                    nc.vector.tensor_copy(ad1[:, t, :], pa[:, 256:260])
                    nc.sync.dma_start(t1stage[t * 128:(t + 1) * 128, :], tab[:])
                    if (t + 1) * 128 == bounds[ci + 1]:
                        ag_chunk(t1stage, t1full, ci)
                        ci += 1

            # ================= PHASE B: L1 edges + L2 head ================
            def edge_phase(layer, tfull, rfw, nh, adt, adrow_tag):
                """One GAT edge phase. Yields per-tile (t, hsb-or-h1T)."""
                halves = (tfull[0:HALF, :], tfull[HALF:2 * HALF, :])
                ncol = nh * HID   # message feature cols (256 / 64)
                rot = (layer == 1)
                maxc = int((CH[:, 0] + CH[:, 1]).max())
                for t in range(NT):
                    ct0 = int(choff[t, 0]); n0 = int(CH[t, 0])
                    ct1 = int(choff[t, 1]); n1 = int(CH[t, 1])
                    ctot = n0 + n1
                    adh = sp.tile([128, nh], dt.float16, tag=adrow_tag + "h")
                    nc.scalar.activation(adh[:], adt[:, t, :], Act.Copy)

                    gb = gp.tile([128, int(CH[:, 0].max() + CH[:, 1].max()), rfw],
                                 dt.float16, tag=f"gb{layer}")
                    for gi, (hoff, nch) in enumerate(((ct0, n0), (ct1, n1))):
                        boff = 0 if gi == 0 else n0
                        nc.gpsimd.dma_gather(
                            gb[:, boff:boff + nch, :], halves[gi],
                            gidx[:, hoff * 8:(hoff + nch) * 8],
                            num_idxs=nch * 128, num_idxs_reg=nch * 128,
                            elem_size=rfw, queue_num=(t * 2 + gi) % 4,
                            single_packet=(nch * 128 <= 1024))
                    # one-hot for all chunks of this tile
                    oh = gp2.tile([128, maxc, 128], dt.float16, tag="oh")
                    dc = dcol[:, ct0:ct0 + ctot]  # groups contiguous per tile
                    nc.vector.tensor_tensor(
                        oh[:, 0:ctot, :],
                        iotam[:].unsqueeze(1).broadcast_to([128, ctot, 128]),
                        dc.unsqueeze(2).broadcast_to([128, ctot, 128]),
                        AluOp.is_equal)
                    # shared psum bank per tile: [agg 0:ncol+nh | ade tail]
                    pbk = ps.tile([128, 440], dt.float32, tag="pagg")
                    pb = pbk[:, 0:ncol + nh]
                    ade = pbk[:, ncol + nh:ncol + nh + maxc * nh].rearrange(
                        "p (c k) -> p c k", k=nh)
                    # one-hot transpose on the PE (8 chunks per psum bank),
                    # copied to SBUF for the alpha_dst matmuls
                    ohT = gp2.tile([128, maxc * 128], dt.float16, tag="ohT")
                    for b0 in range(0, ctot, 8):
                        bn = min(8, ctot - b0)
                        ott = ps.tile([128, 512], dt.float32, tag="ptt")
                        oth = ott[:].bitcast(dt.float16)
                        for j in range(bn):
                            nc.tensor.transpose(oth[:, j * 128:(j + 1) * 128],
                                                oh[:, b0 + j, :], identh[:])
                        nc.scalar.copy(ohT[:, b0 * 128:(b0 + bn) * 128],
                                       oth[:, 0:bn * 128])
                    # alpha_dst per edge: ohT.T @ adh per chunk into psum strip
                    for c in range(ctot):
                        nc.tensor.matmul(
                            ade[:, c, :],
                            ohT[:, c * 128:(c + 1) * 128], adh[:],
                            start=True, stop=True)
                    # e = a_src + a_dst ; lrelu ; exp
                    ee = sp.tile([128, maxc, nh], dt.float32, tag=f"ee{layer}")
                    if rot:
                        as_ap = (gb[:, 0:ctot, :]
                                 .rearrange("p c (h f) -> p c h f", h=nh)
                                 [:, :, :, 0:1].squeeze(3))
                    else:
                        as_ap = gb[:, 0:ctot, ncol:ncol + 2 * nh].bitcast(dt.float32)
                    nc.vector.tensor_tensor(ee[:, 0:ctot, :], as_ap,
                                            ade[:, 0:ctot, :], AluOp.add)
                    nc.vector.scalar_tensor_tensor(
                        ee[:, 0:ctot, :], ee[:, 0:ctot, :], NEG_SLOPE,
                        ee[:, 0:ctot, :], AluOp.mult, AluOp.max)
                    ex = sp.tile([128, maxc, nh], dt.float16, tag=f"ex{layer}")
                    nc.scalar.activation(ex[:, 0:ctot, :], ee[:, 0:ctot, :],
                                         Act.Exp)
                    # scale messages by exp (in place, per head block)
                    nc.vector.tensor_tensor(
                        gb[:, 0:ctot, 0:ncol].rearrange(
                            "p c (h f) -> p c h f", h=nh),
                        gb[:, 0:ctot, 0:ncol].rearrange(
                            "p c (h f) -> p c h f", h=nh),
                        ex[:, 0:ctot, :].unsqueeze(3)
                          .broadcast_to([128, ctot, nh, HID]),
                        AluOp.mult)
                    # aggregate: messages and exp-sums into one psum bank
                    for c in range(ctot):
                        nc.tensor.matmul(pb[:, 0:ncol], oh[:, c, :],
                                         gb[:, c, 0:ncol],
                                         start=(c == 0), stop=(c == ctot - 1))
                        nc.tensor.matmul(pb[:, ncol:ncol + nh], oh[:, c, :],
                                         ex[:, c, :],
                                         start=(c == 0), stop=(c == ctot - 1))
                    # epilogue: h = num / (den + eps), relu (+ unrotation L1)
                    den = sp.tile([128, nh], dt.float32, tag=f"den{layer}")
                    nc.vector.tensor_scalar(den[:], pb[:, ncol:ncol + nh],
                                            1e-16, None, AluOp.add)
                    rec = sp.tile([128, nh], dt.float32, tag=f"rec{layer}")
                    nc.vector.reciprocal(rec[:], den[:])
                    if not rot:
                        hsb = sb.tile([128, ncol], dt.float16, tag=f"h{layer}")
                        nc.scalar.activation(hsb[:], pb[:, 0:ncol], Act.Relu,
                                             scale=rec[:, 0:1])
                        yield t, hsb
                        continue
                    hy = sb.tile([128, ncol], dt.float32, tag="hy")
                    for h in range(nh):
                        nc.scalar.activation(
                            hy[:, h * HID:(h + 1) * HID],
                            pb[:, h * HID:(h + 1) * HID], Act.Copy,
                            scale=rec[:, h:h + 1])
                    # transpose agg_y, unrotate (R = Q^-1 s1), bias+relu -> h1T
                    yT = sb.tile([128, 2, 128], dt.float16, tag="yT")
                    for k in range(2):
                        pt = ps.tile([128, 512], dt.float32, tag="pep")
                        nc.tensor.transpose(pt[:, 0:128],
                                            hy[:, k * 128:(k + 1) * 128],
                                            ident[:])
                        nc.scalar.copy(yT[:, k, :], pt[:, 0:128])
                    h1T = sb.tile([128, 2, 128], dt.float16, tag="h1T")
                    for k in range(2):
                        p2 = ps.tile([128, 512], dt.float32, tag="pep")
                        nc.tensor.matmul(p2[:, 0:128], Rpack[:, k, :],
                                         yT[:, k, :],
                                         start=True, stop=True)
                        nc.scalar.activation(h1T[:, k, :], p2[:, 0:128],
                                             Act.Relu, bias=t1colT[:, k:k + 1])
                    yield t, h1T

            # L1 edge phase; fused L2 head per tile
            ci = 0
            for t, h1T in edge_phase(1, t1full, RF1, H, ad1, "a1"):
                pc = ps.tile([128, 512], dt.float32, tag="pep")
                for k in range(2):
                    nc.tensor.matmul(pc[:, 0:66], h1T[:, k, :], W2e[:, k, :],
                                     start=(k == 0), stop=False)
                nc.tensor.matmul(pc[:, 0:66], ones[0:1, :], t2row[:],
                                 start=False, stop=True)
                tab2 = sb.tile([128, RF2], dt.float16, tag="tab2")
                nc.scalar.activation(tab2[:, 0:HID], pc[:, 0:HID], Act.Copy)
                nc.vector.tensor_copy(tab2[:, HID:HID + 2].bitcast(dt.float32),
                                      pc[:, HID:HID + 1])
                nc.vector.tensor_copy(ad2[:, t, :], pc[:, HID + 1:HID + 2])
                nc.vector.memset(tab2[:, HID + 2:RF2], 0.0)
                nc.sync.dma_start(t2stage[t * 128:(t + 1) * 128, :], tab2[:])
                if (t + 1) * 128 == bounds[ci + 1]:
                    ag_chunk(t2stage, t2full, ci)
                    ci += 1

            # ================= PHASE C: L2 edges + pooling ================
            pgsum = psp.tile([G, HID], dt.float32, tag="pgsum")
            for t, h2 in edge_phase(2, t2full, RF2, 1, ad2, "a2"):
                nc.tensor.matmul(pgsum[:], poh[:, t, :], h2[:],
                                 start=(t == 0), stop=(t == NT - 1))

            # ================= PHASE D: AllReduce + classifier ============
            ar_in = dram.tile([G, HID], dt.float32)
            ar_out = nc.dram_tensor("ar_out", [G, HID], dt.float32,
                                    kind="Internal", addr_space="Shared").ap()
            psum_sb = sb.tile([G, HID], dt.float32, tag="psum_sb")
            nc.vector.tensor_copy(psum_sb[:], pgsum[:])
            nc.sync.dma_start(ar_in[:], psum_sb[:])
            nc.gpsimd.collective_compute(
                "AllReduce", mybir.AluOpType.add, replica_groups=RG,
                ins=[ar_in.opt()], outs=[ar_out.opt()])
            rep = sb.tile([G, HID], dt.float32, tag="rep")
            nc.sync.dma_start(rep[:], ar_out[:])
            nc.vector.tensor_scalar(rep[:], rep[:], cntrecip[:, 0:1], None,
                                    AluOp.mult)
            # hc = relu(rep @ Wc1 + bc1)
            ptr = ps.tile([128, 512], dt.float32, tag="pep")
            nc.tensor.transpose(ptr[0:G, 0:G], rep[:], ident[0:G, 0:G])
            repT = sb.tile([G, G], dt.float32, tag="repT")
            nc.scalar.copy(repT[:], ptr[0:G, 0:G])
            ph = ps.tile([128, 512], dt.float32, tag="pep")
            nc.tensor.matmul(ph[0:G, 0:HID], repT[:], Wc1[:], start=True, stop=False)
            nc.tensor.matmul(ph[0:G, 0:HID], ones[0:1, 0:G], bc1row[:],
                             start=False, stop=True)
            hc = sb.tile([G, HID], dt.float32, tag="hc")
            nc.scalar.activation(hc[:], ph[0:G, 0:HID], Act.Relu)
            pt2 = ps.tile([128, 512], dt.float32, tag="pep")
            nc.tensor.transpose(pt2[0:G, 0:G], hc[:], ident[0:G, 0:G])
            hcT = sb.tile([G, G], dt.float32, tag="hcT")
            nc.scalar.copy(hcT[:], pt2[0:G, 0:G])
            pl = ps.tile([128, 512], dt.float32, tag="pep")
            nc.tensor.matmul(pl[0:G, 0:2], hcT[:], Wc2[:], start=True, stop=False)
            nc.tensor.matmul(pl[0:G, 0:2], ones[0:1, 0:G], bc2row[:],
                             start=False, stop=True)
            # log softmax over the 2 logits
            lg = sb.tile([G, 2], dt.float32, tag="lg")
            nc.vector.tensor_copy(lg[:], pl[0:G, 0:2])
            mx = sb.tile([G, 1], dt.float32, tag="mx")
            nc.vector.tensor_reduce(mx[:], lg[:], mybir.AxisListType.X,
                                    AluOp.max)
            nc.vector.tensor_scalar(lg[:], lg[:], mx[:, 0:1], None,
                                    AluOp.subtract)
            ex = sb.tile([G, 2], dt.float32, tag="ex")
            nc.scalar.activation(ex[:], lg[:], Act.Exp)
            sm = sb.tile([G, 1], dt.float32, tag="sm")
            nc.vector.tensor_reduce(sm[:], ex[:], mybir.AxisListType.X,
                                    AluOp.add)
            ls = sb.tile([G, 1], dt.float32, tag="ls")
            nc.scalar.activation(ls[:], sm[:], Act.Ln)
            outv = sb.tile([G, 2], dt.float32, tag="outv")
            nc.vector.tensor_scalar(outv[:], lg[:], ls[:, 0:1], None,
                                    AluOp.subtract)
            nc.sync.dma_start(out_d[:], outv[:])

    nc.compile()
    return nc


_last_result = [None]


def kernel(**inputs):
    import hashlib
    configure(int(np.asarray(inputs["x"]).shape[0]),
              int(np.asarray(inputs["edge_index"]).shape[1]))
    ek = np.ascontiguousarray(np.asarray(inputs["edge_index"]))
    bk = np.ascontiguousarray(np.asarray(inputs["batch"]))
    key = hashlib.sha1(ek.tobytes() + bk.tobytes()).hexdigest()
    in_maps, CH, choff, CHUNKS = _prep_host(**inputs)
    if key not in _cache:
        _cache[key] = _build(CH, choff, CHUNKS)
    nc = _cache[key]
    res = bass_utils.run_bass_kernel_spmd(nc, in_maps, core_ids=list(range(NC)))
    _last_result[0] = res
    return res.results[0]["out"].astype(np.float32)


def kernel_exec_ns():
    r = _last_result[0]
    return None if r is None else r.exec_time_ns


# revision 33
# speedup vs baseline: 1.7027x; 1.0865x over previous
"""GAT (2-layer, 4->1 heads) + global mean pool + classifier on 8 trn2 NeuronCores.

Sharding: nodes (and their incoming edges) partitioned contiguously across the
8 cores; small weights replicated; per-layer node-feature tables AllGathered
(chunked, overlapped with producer compute); per-graph pooled sums AllReduced.

Layer-1 table rows are rotated per head (first basis vector = att_src) so the
512B row carries both the message features and alpha_src; the inverse rotation
is folded into the transposed epilogue feeding the layer-2 head. The edge
one-hot transpose (for the alpha_dst lookup) is done on the PE instead of a
broadcast DMA + compare.

Self-contained: takes full inputs, returns full [64, 2] log-softmax output.
"""
import sys
for _p in ('/opt/trn_rl_repo', '/root/.axon_site/_ro/trn_rl_repo'):
    if _p not in sys.path:
        sys.path.insert(0, _p)

import numpy as np
import concourse.bass as bass
import concourse.bacc as bacc
import concourse.tile as tile
import concourse.mybir as mybir
from concourse import bass_utils, library_config

dt = mybir.dt

# problem constants (hardcoded per contract); N/E re-derivable for sim tests
N = 50000
E = 1600000
G = 64
DIN = 128
HID = 64
H = 4
NEG_SLOPE = 0.2
BN_EPS = 1e-5
NC = 8
RF1 = 256                # L1 table row: fp16 slots (512B): y = xw @ Q (as = y[h,0])
RF2 = 128                # L2 table row: fp16 slots (256B): 64 xws | 2 a_src(f32) | pad


def configure(n, e):
    global N, E, NSH, NT, NPAD, HALF
    N, E = n, e
    NSH = N // NC
    NT = (NSH + 127) // 128
    NPAD = NT * 128
    HALF = NC // 2 * NPAD


configure(N, E)

_cache = {}


def _prep_host(x, edge_index, batch,
               W1, att_src1, att_dst1, bias1, bn1_g, bn1_b, bn1_m, bn1_v,
               W2, att_src2, att_dst2, bias2, bn2_g, bn2_b, bn2_m, bn2_v,
               Wc1, bc1, Wc2, bc2):
    """Index-space layout + folded weights. Returns (in_maps, CH, meta)."""
    f32 = np.float32
    src = np.concatenate([np.asarray(edge_index[0], np.int64),
                          np.arange(N, dtype=np.int64)])
    dst = np.concatenate([np.asarray(edge_index[1], np.int64),
                          np.arange(N, dtype=np.int64)])
    EE = src.shape[0]

    # relabel nodes: snake-assign by in-degree so every (core, tile) bucket
    # has a near-equal edge count (balances chunk counts across cores)
    indeg = np.bincount(dst, minlength=N)
    order_by_deg = np.argsort(-indeg, kind='stable')
    nbuckets = NC * NT
    newid = np.empty(N, np.int64)
    bidx = np.arange(N) % nbuckets
    snake = (np.arange(N) // nbuckets) % 2 == 1
    bidx = np.where(snake, nbuckets - 1 - bidx, bidx)
    # bucket b corresponds to core b // NT, tile b % NT
    slot_in_bucket = np.zeros(N, np.int64)
    counts = np.zeros(nbuckets, np.int64)
    for i in range(N):
        b = bidx[i]
        while counts[b] >= 128:
            b = (b + 1) % nbuckets
        slot_in_bucket[i] = counts[b]
        counts[b] += 1
        bidx[i] = b
    cores_of = bidx // NT
    tiles_of = bidx % NT
    newid[order_by_deg] = cores_of * NSH + tiles_of * 128 + slot_in_bucket
    # tile NT-1 slots beyond NSH are ghosts; ensure none assigned
    lastcap = NSH - (NT - 1) * 128
    bad = (tiles_of == NT - 1) & (slot_in_bucket >= lastcap)
    if bad.any():
        ov = np.where(bad)[0]
        free_buckets = [b for b in range(nbuckets)
                        if (b % NT != NT - 1 and counts[b] < 128)
                        or (b % NT == NT - 1 and counts[b] < lastcap)]
        fi = 0
        for i in ov:
            while True:
                b = free_buckets[fi % len(free_buckets)]
                cap = 128 if b % NT != NT - 1 else lastcap
                if counts[b] < cap:
                    break
                fi += 1
            slot_in_bucket[i] = counts[b]
            counts[b] += 1
            cores_of[i] = b // NT
            tiles_of[i] = b % NT
            fi += 1
        newid[order_by_deg] = cores_of * NSH + tiles_of * 128 + slot_in_bucket
    inv = np.empty(N, np.int64)
    inv[newid] = np.arange(N)      # inv[new] = old
    src = newid[src]
    dst = newid[dst]

    core = dst // NSH
    ldst = dst - core * NSH
    t = ldst >> 7                     # dst tile within shard
    dit = ldst & 127                  # dst index within tile
    # table rows laid out chunk-major (per allgather chunk, core-major
    # inside) so each chunk's collective output is contiguous
    AGC = 4
    rowb = np.array([(i * NT // AGC) * 128 for i in range(AGC + 1)])
    cumout = np.concatenate([[0], np.cumsum(NC * np.diff(rowb))])
    ls = src % NSH
    ci = np.searchsorted(rowb, ls, side='right') - 1
    trow = (cumout[ci] + (src // NSH) * (rowb[ci + 1] - rowb[ci])
            + (ls - rowb[ci]))
    g = (trow >= HALF).astype(np.int64)           # table half by row
    lidx = trow - g * HALF                        # int16-safe local row

    key = ((core * NT + t) * 2 + g)   # bucket id, core-major
    nbuck = NC * NT * 2
    cnt = np.bincount(key, minlength=nbuck).reshape(NC, NT, 2)
    CH = np.maximum(1, (cnt.max(axis=0) + 127) // 128)   # [NT, 2] shared chunks
    CH = CH + (CH & 1)   # even counts -> 4B-aligned stream slice offsets
    CHUNKS = int(CH.sum())
    choff = np.zeros((NT, 2), np.int64)
    choff.reshape(-1)[1:] = np.cumsum(CH.reshape(-1))[:-1]

    # stable-sort edges by bucket; ranks within bucket
    order = np.argsort(key, kind='stable')
    skey = key[order]
    bstart = np.searchsorted(skey, np.arange(nbuck))
    rank = np.arange(EE, dtype=np.int64) - bstart[skey]
    # padded stream position (per core stream of CHUNKS*128 slots)
    bt = (skey // 2) % NT
    bg = skey % 2
    pos = choff[bt, bg] * 128 + rank
    scor = skey // (NT * 2)

    gidx_all = np.zeros((NC, CHUNKS * 128), np.int16)
    dcol_all = np.full((NC, CHUNKS * 128), 999.0, np.float16)
    for c in range(NC):
        m = scor == c
        eidx = order[m]
        gidx_all[c, pos[m]] = lidx[eidx].astype(np.int16)
        dcol_all[c, pos[m]] = dit[eidx].astype(np.float16)

    # wrap: element i -> [i % 16, i // 16] / dstcol: chunk-major -> [128, CHUNKS]
    gidx = np.tile(gidx_all.reshape(NC, CHUNKS * 8, 16).transpose(0, 2, 1), (1, 8, 1)).copy()
    dcol = dcol_all.reshape(NC, CHUNKS, 128).transpose(0, 2, 1).copy()

    # batch / pooling (note: node n' holds old node inv[n'])
    batch = np.asarray(batch, np.int64)[inv]
    bcol = np.full((NC, 128, NT), 999.0, np.float16)
    for c in range(NC):
        bc_ = batch[c * NSH:(c + 1) * NSH].astype(np.float16)
        pad = np.full(NPAD - NSH, 999.0, np.float16)
        bcol[c] = np.concatenate([bc_, pad]).reshape(NT, 128).T
    cnt_g = np.bincount(batch, minlength=G).astype(f32)
    cntrecip = (1.0 / np.maximum(cnt_g, 1.0)).reshape(G, 1)

    # folded weights
    W1 = np.asarray(W1, f32); W2 = np.asarray(W2, f32)
    s1 = np.asarray(bn1_g, f32) / np.sqrt(np.asarray(bn1_v, f32) + BN_EPS)
    t1 = (np.asarray(bias1, f32) - np.asarray(bn1_m, f32)) * s1 + np.asarray(bn1_b, f32)
    s2 = np.asarray(bn2_g, f32) / np.sqrt(np.asarray(bn2_v, f32) + BN_EPS)
    t2 = (np.asarray(bias2, f32) - np.asarray(bn2_m, f32)) * s2 + np.asarray(bn2_b, f32)
    aS1 = np.asarray(att_src1, f32)   # [H, HID]
    aD1 = np.asarray(att_dst1, f32)
    # per-head rotation Q (first column = att_src) and inverse R = Q^-1 * s1
    rng = np.random.default_rng(12345)
    Qblk = np.zeros((H * HID, H * HID), f32)
    Rpack = np.zeros((128, 2, 128), f32)     # blockdiag pairs of R_h
    for h in range(H):
        a = aS1[h]
        M = np.concatenate([a[:, None],
                            rng.standard_normal((HID, HID - 1)).astype(f32)], 1)
        Qf, _ = np.linalg.qr(M)
        Q = np.concatenate([a[:, None], Qf[:, 1:]], 1).astype(f32)
        R = np.linalg.inv(Q).astype(f32) * s1[h * HID:(h + 1) * HID][None, :]
        Qblk[h * HID:(h + 1) * HID, h * HID:(h + 1) * HID] = Q
        b2, r2_ = divmod(h, 2)
        Rpack[r2_ * HID:(r2_ + 1) * HID, b2, r2_ * HID:(r2_ + 1) * HID] = R
    AblkD = np.zeros((H * HID, H), f32)
    for h in range(H):
        AblkD[h * HID:(h + 1) * HID, h] = aD1[h]
    W1e = np.concatenate([W1 @ Qblk, W1 @ AblkD], axis=1)            # [128, 260]
    t1colT = t1.reshape(2, 128).T.copy()                             # [128, 2]
    aS2 = np.asarray(att_src2, f32).reshape(HID)
    aD2 = np.asarray(att_dst2, f32).reshape(HID)
    W2e = np.concatenate([W2 * s2[None, :], (W2 @ aS2)[:, None],
                          (W2 @ aD2)[:, None]], axis=1)              # [256, 66]
    t2row = np.concatenate([t2, np.zeros(2, f32)]).reshape(1, 66)

    iotam = np.tile(np.arange(128, dtype=np.float16), (128, 1))
    iota64 = np.tile(np.arange(64, dtype=np.float16), (128, 1))
    ident = np.eye(128, dtype=f32)
    identh = np.eye(128, dtype=np.float16)
    onesrow = np.ones((1, 128), f32)

    x = np.asarray(x, f32)[inv]
    in_maps = []
    for c in range(NC):
        xs = x[c * NSH:(c + 1) * NSH]
        xT = np.zeros((DIN, NPAD), np.float16)
        xT[:, :NSH] = xs.T.astype(np.float16)
        in_maps.append({
            "xT": xT, "gidx": gidx[c], "dcol": dcol[c],
            "bcol": bcol[c].copy(),
            "W1e": W1e.astype(np.float16), "t1colT": t1colT,
            "Rpack": Rpack.astype(np.float16),
            "W2e": W2e.reshape(2, 128, 66).transpose(1, 0, 2).astype(np.float16).copy(),
            "t2row": t2row,
            "iotam": iotam, "iota64": iota64, "ident": ident, "identh": identh,
            "onesrow": onesrow,
            "Wc1": np.asarray(Wc1, f32), "bc1row": np.asarray(bc1, f32).reshape(1, HID),
            "Wc2": np.asarray(Wc2, f32), "bc2row": np.asarray(bc2, f32).reshape(1, 2),
            "cntrecip": cntrecip,
        })
    return in_maps, CH, choff, CHUNKS


def _build(CH, choff, CHUNKS):
    AluOp = mybir.AluOpType
    Act = mybir.ActivationFunctionType
    nc = bacc.Bacc("TRN2", target_bir_lowering=False, debug=False, num_devices=NC,
                   num_swdge_queues=4)

    xT_d = nc.dram_tensor("xT", [DIN, NPAD], dt.float16, kind="ExternalInput")
    gidx_d = nc.dram_tensor("gidx", [128, CHUNKS * 8], dt.int16, kind="ExternalInput")
    dcol_d = nc.dram_tensor("dcol", [128, CHUNKS], dt.float16, kind="ExternalInput")
    bcol_d = nc.dram_tensor("bcol", [128, NT], dt.float16, kind="ExternalInput")
    W1e_d = nc.dram_tensor("W1e", [DIN, 260], dt.float16, kind="ExternalInput")
    t1colT_d = nc.dram_tensor("t1colT", [128, 2], dt.float32, kind="ExternalInput")
    Rpack_d = nc.dram_tensor("Rpack", [128, 2, 128], dt.float16, kind="ExternalInput")
    W2e_d = nc.dram_tensor("W2e", [128, 2, 66], dt.float16, kind="ExternalInput")
    t2row_d = nc.dram_tensor("t2row", [1, 66], dt.float32, kind="ExternalInput")
    iotam_d = nc.dram_tensor("iotam", [128, 128], dt.float16, kind="ExternalInput")
    iota64_d = nc.dram_tensor("iota64", [128, 64], dt.float16, kind="ExternalInput")
    ident_d = nc.dram_tensor("ident", [128, 128], dt.float32, kind="ExternalInput")
    identh_d = nc.dram_tensor("identh", [128, 128], dt.float16, kind="ExternalInput")
    ones_d = nc.dram_tensor("onesrow", [1, 128], dt.float32, kind="ExternalInput")
    Wc1_d = nc.dram_tensor("Wc1", [HID, HID], dt.float32, kind="ExternalInput")
    bc1_d = nc.dram_tensor("bc1row", [1, HID], dt.float32, kind="ExternalInput")
    Wc2_d = nc.dram_tensor("Wc2", [HID, 2], dt.float32, kind="ExternalInput")
    bc2_d = nc.dram_tensor("bc2row", [1, 2], dt.float32, kind="ExternalInput")
    crec_d = nc.dram_tensor("cntrecip", [G, 1], dt.float32, kind="ExternalInput")
    out_d = nc.dram_tensor("out", [G, 2], dt.float32, kind="ExternalOutput")

    RG = [list(range(NC))]
    AGC = 4                      # allgather chunks per table
    bounds = [(i * NT // AGC) * 128 for i in range(AGC + 1)]

    with tile.TileContext(nc) as tc:
        with (
            tc.tile_pool(name="const", bufs=1) as cp,
            tc.tile_pool(name="sb", bufs=3) as sb,
            tc.tile_pool(name="gbuf", bufs=3) as gp,
            tc.tile_pool(name="gbuf2", bufs=3) as gp2,
            tc.tile_pool(name="small", bufs=4) as sp,
            tc.tile_pool(name="ps", bufs=2, space="PSUM") as ps,
            tc.tile_pool(name="pspool", bufs=1, space="PSUM") as psp,
            tc.tile_pool(name="dram", bufs=1, space="DRAM") as dram,
        ):
            nc.gpsimd.load_library(library_config.mlp)

            # ---- consts to SBUF
            def cload(dten, shape, dtype):
                tl = cp.tile(shape, dtype, tag=dten.name)
                nc.sync.dma_start(tl[:], dten[:])
                return tl
            W1e = cload(W1e_d, [DIN, 260], dt.float16)
            t1colT = cload(t1colT_d, [128, 2], dt.float32)
            Rpack = cload(Rpack_d, [128, 2, 128], dt.float16)
            W2e = cload(W2e_d, [128, 2, 66], dt.float16)
            t2row = cload(t2row_d, [1, 66], dt.float32)
            iotam = cload(iotam_d, [128, 128], dt.float16)
            iota64 = cload(iota64_d, [128, 64], dt.float16)
            ident = cload(ident_d, [128, 128], dt.float32)
            identh = cload(identh_d, [128, 128], dt.float16)
            ones = cload(ones_d, [1, 128], dt.float32)
            Wc1 = cload(Wc1_d, [HID, HID], dt.float32)
            bc1row = cload(bc1_d, [1, HID], dt.float32)
            Wc2 = cload(Wc2_d, [HID, 2], dt.float32)
            bc2row = cload(bc2_d, [1, 2], dt.float32)
            cntrecip = cload(crec_d, [G, 1], dt.float32)
            gidx = cload(gidx_d, [128, CHUNKS * 8], dt.int16)
            dcol = cload(dcol_d, [128, CHUNKS], dt.float16)
            bcol = cload(bcol_d, [128, NT], dt.float16)

            ad1 = cp.tile([128, NT, H], dt.float32, tag="ad1")
            ad2 = cp.tile([128, NT, 1], dt.float32, tag="ad2")
            poh = cp.tile([128, NT, G], dt.float16, tag="poh")
            # pooling one-hot (built once)
            nc.vector.tensor_tensor(
                poh[:],
                iota64[:].unsqueeze(1).broadcast_to([128, NT, G]),
                bcol[:].unsqueeze(2).broadcast_to([128, NT, G]),
                AluOp.is_equal)

            # ---- DRAM tables (collective outputs in Shared space)
            t1stage = dram.tile([NPAD, RF1], dt.float16)
            t1full = nc.dram_tensor("t1full", [NC * NPAD, RF1], dt.float16,
                                    kind="Internal", addr_space="Shared").ap()
            t2stage = dram.tile([NPAD, RF2], dt.float16)
            t2full = nc.dram_tensor("t2full", [NC * NPAD, RF2], dt.float16,
                                    kind="Internal", addr_space="Shared").ap()

            def ag_chunk(stage, full, ci):
                r0, r1 = bounds[ci], bounds[ci + 1]
                o0, o1 = NC * r0, NC * r1
                nc.gpsimd.collective_compute(
                    "AllGather", mybir.AluOpType.bypass, replica_groups=RG,
                    ins=[stage[r0:r1, :].opt()],
                    outs=[full[o0:o1, :].opt()])

            # ================= PHASE A: L1 head (y table + alpha_dst) =====
            with nc.named_scope("phaseA"), tc.tile_pool(name="head", bufs=3) as hp:
                ci = 0
                for t in range(NT):
                    xTt = hp.tile([DIN, 128], dt.float16, tag="xTt")
                    nc.sync.dma_start(xTt[:], xT_d[:, t * 128:(t + 1) * 128])
                    pa = ps.tile([128, 512], dt.float32, tag="pep")
                    nc.tensor.matmul(pa[:, 0:260], xTt[:], W1e[:],
                                     start=True, stop=True)
                    tab = sb.tile([128, RF1], dt.float16, tag="tab1")
                    nc.scalar.activation(tab[:], pa[:, 0:256], Act.Copy)
                    nc.vector.tensor_copy(ad1[:, t, :], pa[:, 256:260])
                    nc.sync.dma_start(t1stage[t * 128:(t + 1) * 128, :], tab[:])
                    if (t + 1) * 128 == bounds[ci + 1]:
                        ag_chunk(t1stage, t1full, ci)
                        ci += 1

            # ================= PHASE B: L1 edges + L2 head ================
            def edge_phase(layer, tfull, rfw, nh, adt, adrow_tag):
                """One GAT edge phase. Yields per-tile (t, hsb-or-h1T)."""
                halves = (tfull[0:HALF, :], tfull[HALF:2 * HALF, :])
                ncol = nh * HID   # message feature cols (256 / 64)
                rot = (layer == 1)
                maxc = int((CH[:, 0] + CH[:, 1]).max())
                for t in range(NT):
                    ct0 = int(choff[t, 0]); n0 = int(CH[t, 0])
                    ct1 = int(choff[t, 1]); n1 = int(CH[t, 1])
                    ctot = n0 + n1
                    adh = sp.tile([128, nh], dt.float16, tag=adrow_tag + "h")
                    nc.scalar.activation(adh[:], adt[:, t, :], Act.Copy)

                    gb = gp.tile([128, int(CH[:, 0].max() + CH[:, 1].max()), rfw],
                                 dt.float16, tag=f"gb{layer}")
                    for gi, (hoff, nch) in enumerate(((ct0, n0), (ct1, n1))):
                        boff = 0 if gi == 0 else n0
                        nc.gpsimd.dma_gather(
                            gb[:, boff:boff + nch, :], halves[gi],
                            gidx[:, hoff * 8:(hoff + nch) * 8],
                            num_idxs=nch * 128, num_idxs_reg=nch * 128,
                            elem_size=rfw, queue_num=(t * 2 + gi) % 4,
                            single_packet=(nch * 128 <= 1024))
                    # one-hot for all chunks of this tile
                    oh = gp2.tile([128, maxc, 128], dt.float16, tag="oh")
                    dc = dcol[:, ct0:ct0 + ctot]  # groups contiguous per tile
                    nc.vector.tensor_tensor(
                        oh[:, 0:ctot, :],
                        iotam[:].unsqueeze(1).broadcast_to([128, ctot, 128]),
                        dc.unsqueeze(2).broadcast_to([128, ctot, 128]),
                        AluOp.is_equal)
                    # shared psum bank per tile: [agg 0:ncol+nh | ade tail]
                    pbk = ps.tile([128, 440], dt.float32, tag="pagg")
                    pb = pbk[:, 0:ncol + nh]
                    ade = pbk[:, ncol + nh:ncol + nh + maxc * nh].rearrange(
                        "p (c k) -> p c k", k=nh)
                    # one-hot transpose on the PE (8 chunks per psum bank),
                    # copied to SBUF for the alpha_dst matmuls
                    ohT = gp2.tile([128, maxc * 128], dt.float16, tag="ohT")
                    for b0 in range(0, ctot, 8):
                        bn = min(8, ctot - b0)
                        ott = ps.tile([128, 512], dt.float32, tag="ptt")
                        oth = ott[:].bitcast(dt.float16)
                        for j in range(bn):
                            nc.tensor.transpose(oth[:, j * 128:(j + 1) * 128],
                                                oh[:, b0 + j, :], identh[:])
                        nc.scalar.copy(ohT[:, b0 * 128:(b0 + bn) * 128],
                                       oth[:, 0:bn * 128])
                    # alpha_dst per edge: ohT.T @ adh per chunk into psum strip
                    for c in range(ctot):
                        nc.tensor.matmul(
                            ade[:, c, :],
                            ohT[:, c * 128:(c + 1) * 128], adh[:],
                            start=True, stop=True)
                    # e = a_src + a_dst ; lrelu ; exp
                    ee = sp.tile([128, maxc, nh], dt.float32, tag=f"ee{layer}")
                    if rot:
                        as_ap = (gb[:, 0:ctot, :]
                                 .rearrange("p c (h f) -> p c h f", h=nh)
                                 [:, :, :, 0:1].squeeze(3))
                    else:
                        as_ap = gb[:, 0:ctot, ncol:ncol + 2 * nh].bitcast(dt.float32)
                    nc.vector.tensor_tensor(ee[:, 0:ctot, :], as_ap,
                                            ade[:, 0:ctot, :], AluOp.add)
                    nc.vector.scalar_tensor_tensor(
                        ee[:, 0:ctot, :], ee[:, 0:ctot, :], NEG_SLOPE,
                        ee[:, 0:ctot, :], AluOp.mult, AluOp.max)
                    ex = sp.tile([128, maxc, nh], dt.float16, tag=f"ex{layer}")
                    nc.scalar.activation(ex[:, 0:ctot, :], ee[:, 0:ctot, :],
                                         Act.Exp)
                    # scale messages by exp (in place, per head block)
                    nc.vector.tensor_tensor(
                        gb[:, 0:ctot, 0:ncol].rearrange(
                            "p c (h f) -> p c h f", h=nh),
                        gb[:, 0:ctot, 0:ncol].rearrange(
                            "p c (h f) -> p c h f", h=nh),
                        ex[:, 0:ctot, :].unsqueeze(3)
                          .broadcast_to([128, ctot, nh, HID]),
                        AluOp.mult)
                    # aggregate: messages and exp-sums into one psum bank
                    for c in range(ctot):
                        nc.tensor.matmul(pb[:, 0:ncol], oh[:, c, :],
                                         gb[:, c, 0:ncol],
                                         start=(c == 0), stop=(c == ctot - 1))
                        nc.tensor.matmul(pb[:, ncol:ncol + nh], oh[:, c, :],
                                         ex[:, c, :],
                                         start=(c == 0), stop=(c == ctot - 1))
                    # epilogue: h = num / (den + eps), relu (+ unrotation L1)
                    den = sp.tile([128, nh], dt.float32, tag=f"den{layer}")
                    nc.vector.tensor_scalar(den[:], pb[:, ncol:ncol + nh],
                                            1e-16, None, AluOp.add)
                    rec = sp.tile([128, nh], dt.float32, tag=f"rec{layer}")
                    nc.vector.reciprocal(rec[:], den[:])
                    if not rot:
                        hsb = sb.tile([128, ncol], dt.float16, tag=f"h{layer}")
                        nc.scalar.activation(hsb[:], pb[:, 0:ncol], Act.Relu,
                                             scale=rec[:, 0:1])
                        yield t, hsb
                        continue
                    hy = sb.tile([128, ncol], dt.float32, tag="hy")
                    for h in range(nh):
                        nc.scalar.activation(
                            hy[:, h * HID:(h + 1) * HID],
                            pb[:, h * HID:(h + 1) * HID], Act.Copy,
                            scale=rec[:, h:h + 1])
                    # transpose agg_y, unrotate (R = Q^-1 s1), bias+relu -> h1T
                    yT = sb.tile([128, 2, 128], dt.float16, tag="yT")
                    for k in range(2):
                        pt = ps.tile([128, 512], dt.float32, tag="pep")
                        nc.tensor.transpose(pt[:, 0:128],
                                            hy[:, k * 128:(k + 1) * 128],
                                            ident[:])
                        nc.scalar.copy(yT[:, k, :], pt[:, 0:128])
                    h1T = sb.tile([128, 2, 128], dt.float16, tag="h1T")
                    for k in range(2):
                        p2 = ps.tile([128, 512], dt.float32, tag="pep")
                        nc.tensor.matmul(p2[:, 0:128], Rpack[:, k, :],
                                         yT[:, k, :],
                                         start=True, stop=True)
                        nc.scalar.activation(h1T[:, k, :], p2[:, 0:128],
                                             Act.Relu, bias=t1colT[:, k:k + 1])
                    yield t, h1T

            # L1 edge phase; fused L2 head per tile
            ci = 0
            for t, h1T in edge_phase(1, t1full, RF1, H, ad1, "a1"):
                pc = ps.tile([128, 512], dt.float32, tag="pep")
                for k in range(2):
                    nc.tensor.matmul(pc[:, 0:66], h1T[:, k, :], W2e[:, k, :],
                                     start=(k == 0), stop=False)
                nc.tensor.matmul(pc[:, 0:66], ones[0:1, :], t2row[:],
                                 start=False, stop=True)
                tab2 = sb.tile([128, RF2], dt.float16, tag="tab2")
                nc.scalar.activation(tab2[:, 0:HID], pc[:, 0:HID], Act.Copy)
                nc.vector.tensor_copy(tab2[:, HID:HID + 2].bitcast(dt.float32),
                                      pc[:, HID:HID + 1])
                nc.vector.tensor_copy(ad2[:, t, :], pc[:, HID + 1:HID + 2])
                nc.vector.memset(tab2[:, HID + 2:RF2], 0.0)
                nc.sync.dma_start(t2stage[t * 128:(t + 1) * 128, :], tab2[:])
                if (t + 1) * 128 == bounds[ci + 1]:
                    ag_chunk(t2stage, t2full, ci)
                    ci += 1

            # ================= PHASE C: L2 edges + pooling ================
            pgsum = psp.tile([G, HID], dt.float32, tag="pgsum")
            for t, h2 in edge_phase(2, t2full, RF2, 1, ad2, "a2"):
                nc.tensor.matmul(pgsum[:], poh[:, t, :], h2[:],
                                 start=(t == 0), stop=(t == NT - 1))

            # ================= PHASE D: AllReduce + classifier ============
            ar_in = dram.tile([G, HID], dt.float32)
            ar_out = nc.dram_tensor("ar_out", [G, HID], dt.float32,
                                    kind="Internal", addr_space="Shared").ap()
            psum_sb = sb.tile([G, HID], dt.float32, tag="psum_sb")
            nc.vector.tensor_copy(psum_sb[:], pgsum[:])
            nc.sync.dma_start(ar_in[:], psum_sb[:])
            nc.gpsimd.collective_compute(
                "AllReduce", mybir.AluOpType.add, replica_groups=RG,
                ins=[ar_in.opt()], outs=[ar_out.opt()])
            rep = sb.tile([G, HID], dt.float32, tag="rep")
            nc.sync.dma_start(rep[:], ar_out[:])
            nc.vector.tensor_scalar(rep[:], rep[:], cntrecip[:, 0:1], None,
                                    AluOp.mult)
            # hc = relu(rep @ Wc1 + bc1)
            ptr = ps.tile([128, 512], dt.float32, tag="pep")
            nc.tensor.transpose(ptr[0:G, 0:G], rep[:], ident[0:G, 0:G])
            repT = sb.tile([G, G], dt.float32, tag="repT")
            nc.scalar.copy(repT[:], ptr[0:G, 0:G])
            ph = ps.tile([128, 512], dt.float32, tag="pep")
            nc.tensor.matmul(ph[0:G, 0:HID], repT[:], Wc1[:], start=True, stop=False)
            nc.tensor.matmul(ph[0:G, 0:HID], ones[0:1, 0:G], bc1row[:],
                             start=False, stop=True)
            hc = sb.tile([G, HID], dt.float32, tag="hc")
            nc.scalar.activation(hc[:], ph[0:G, 0:HID], Act.Relu)
            pt2 = ps.tile([128, 512], dt.float32, tag="pep")
            nc.tensor.transpose(pt2[0:G, 0:G], hc[:], ident[0:G, 0:G])
            hcT = sb.tile([G, G], dt.float32, tag="hcT")
            nc.scalar.copy(hcT[:], pt2[0:G, 0:G])
            pl = ps.tile([128, 512], dt.float32, tag="pep")
            nc.tensor.matmul(pl[0:G, 0:2], hcT[:], Wc2[:], start=True, stop=False)
            nc.tensor.matmul(pl[0:G, 0:2], ones[0:1, 0:G], bc2row[:],
                             start=False, stop=True)
            # log softmax over the 2 logits
            lg = sb.tile([G, 2], dt.float32, tag="lg")
            nc.vector.tensor_copy(lg[:], pl[0:G, 0:2])
            mx = sb.tile([G, 1], dt.float32, tag="mx")
            nc.vector.tensor_reduce(mx[:], lg[:], mybir.AxisListType.X,
                                    AluOp.max)
            nc.vector.tensor_scalar(lg[:], lg[:], mx[:, 0:1], None,
                                    AluOp.subtract)
            ex = sb.tile([G, 2], dt.float32, tag="ex")
            nc.scalar.activation(ex[:], lg[:], Act.Exp)
            sm = sb.tile([G, 1], dt.float32, tag="sm")
            nc.vector.tensor_reduce(sm[:], ex[:], mybir.AxisListType.X,
                                    AluOp.add)
            ls = sb.tile([G, 1], dt.float32, tag="ls")
            nc.scalar.activation(ls[:], sm[:], Act.Ln)
            outv = sb.tile([G, 2], dt.float32, tag="outv")
            nc.vector.tensor_scalar(outv[:], lg[:], ls[:, 0:1], None,
                                    AluOp.subtract)
            nc.sync.dma_start(out_d[:], outv[:])

    nc.compile()
    return nc


_last_result = [None]


def kernel(**inputs):
    import hashlib
    configure(int(np.asarray(inputs["x"]).shape[0]),
              int(np.asarray(inputs["edge_index"]).shape[1]))
    ek = np.ascontiguousarray(np.asarray(inputs["edge_index"]))
    bk = np.ascontiguousarray(np.asarray(inputs["batch"]))
    key = hashlib.sha1(ek.tobytes() + bk.tobytes()).hexdigest()
    in_maps, CH, choff, CHUNKS = _prep_host(**inputs)
    if key not in _cache:
        _cache[key] = _build(CH, choff, CHUNKS)
    nc = _cache[key]
    res = bass_utils.run_bass_kernel_spmd(nc, in_maps, core_ids=list(range(NC)))
    _last_result[0] = res
    return res.results[0]["out"].astype(np.float32)


def kernel_exec_ns():
    r = _last_result[0]
    return None if r is None else r.exec_time_ns


# revision 39
# speedup vs baseline: 1.7047x; 1.0012x over previous
"""GAT (2-layer, 4->1 heads) + global mean pool + classifier on 8 trn2 NeuronCores.

Sharding: nodes (and their incoming edges) partitioned contiguously across the
8 cores; small weights replicated; per-layer node-feature tables AllGathered
(chunked, overlapped with producer compute); per-graph pooled sums AllReduced.

Layer-1 table rows are rotated per head (first basis vector = att_src) so the
512B row carries both the message features and alpha_src; the inverse rotation
is folded into the transposed epilogue feeding the layer-2 head. The edge
one-hot transpose (for the alpha_dst lookup) is done on the PE instead of a
broadcast DMA + compare.

Self-contained: takes full inputs, returns full [64, 2] log-softmax output.
"""
import sys
for _p in ('/opt/trn_rl_repo', '/root/.axon_site/_ro/trn_rl_repo'):
    if _p not in sys.path:
        sys.path.insert(0, _p)

import numpy as np
import concourse.bass as bass
import concourse.bacc as bacc
import concourse.tile as tile
import concourse.mybir as mybir
from concourse import bass_utils, library_config

dt = mybir.dt

# problem constants (hardcoded per contract); N/E re-derivable for sim tests
N = 50000
E = 1600000
G = 64
DIN = 128
HID = 64
H = 4
NEG_SLOPE = 0.2
BN_EPS = 1e-5
NC = 8
RF1 = 256                # L1 table row: fp16 slots (512B): y = xw @ Q (as = y[h,0])
RF2 = 128                # L2 table row: fp16 slots (256B): 64 xws | 2 a_src(f32) | pad


def configure(n, e):
    global N, E, NSH, NT, NPAD, HALF
    N, E = n, e
    NSH = N // NC
    NT = (NSH + 127) // 128
    NPAD = NT * 128
    HALF = NC // 2 * NPAD


configure(N, E)

_cache = {}


def _prep_host(x, edge_index, batch,
               W1, att_src1, att_dst1, bias1, bn1_g, bn1_b, bn1_m, bn1_v,
               W2, att_src2, att_dst2, bias2, bn2_g, bn2_b, bn2_m, bn2_v,
               Wc1, bc1, Wc2, bc2):
    """Index-space layout + folded weights. Returns (in_maps, CH, meta)."""
    f32 = np.float32
    src = np.concatenate([np.asarray(edge_index[0], np.int64),
                          np.arange(N, dtype=np.int64)])
    dst = np.concatenate([np.asarray(edge_index[1], np.int64),
                          np.arange(N, dtype=np.int64)])
    EE = src.shape[0]

    # relabel nodes: snake-assign by in-degree so every (core, tile) bucket
    # has a near-equal edge count (balances chunk counts across cores)
    indeg = np.bincount(dst, minlength=N)
    order_by_deg = np.argsort(-indeg, kind='stable')
    nbuckets = NC * NT
    newid = np.empty(N, np.int64)
    bidx = np.arange(N) % nbuckets
    snake = (np.arange(N) // nbuckets) % 2 == 1
    bidx = np.where(snake, nbuckets - 1 - bidx, bidx)
    # bucket b corresponds to core b // NT, tile b % NT
    slot_in_bucket = np.zeros(N, np.int64)
    counts = np.zeros(nbuckets, np.int64)
    for i in range(N):
        b = bidx[i]
        while counts[b] >= 128:
            b = (b + 1) % nbuckets
        slot_in_bucket[i] = counts[b]
        counts[b] += 1
        bidx[i] = b
    cores_of = bidx // NT
    tiles_of = bidx % NT
    newid[order_by_deg] = cores_of * NSH + tiles_of * 128 + slot_in_bucket
    # tile NT-1 slots beyond NSH are ghosts; ensure none assigned
    lastcap = NSH - (NT - 1) * 128
    bad = (tiles_of == NT - 1) & (slot_in_bucket >= lastcap)
    if bad.any():
        ov = np.where(bad)[0]
        free_buckets = [b for b in range(nbuckets)
                        if (b % NT != NT - 1 and counts[b] < 128)
                        or (b % NT == NT - 1 and counts[b] < lastcap)]
        fi = 0
        for i in ov:
            while True:
                b = free_buckets[fi % len(free_buckets)]
                cap = 128 if b % NT != NT - 1 else lastcap
                if counts[b] < cap:
                    break
                fi += 1
            slot_in_bucket[i] = counts[b]
            counts[b] += 1
            cores_of[i] = b // NT
            tiles_of[i] = b % NT
            fi += 1
        newid[order_by_deg] = cores_of * NSH + tiles_of * 128 + slot_in_bucket
    inv = np.empty(N, np.int64)
    inv[newid] = np.arange(N)      # inv[new] = old
    src = newid[src]
    dst = newid[dst]

    core = dst // NSH
    ldst = dst - core * NSH
    t = ldst >> 7                     # dst tile within shard
    dit = ldst & 127                  # dst index within tile
    # table rows laid out chunk-major (per allgather chunk, core-major
    # inside) so each chunk's collective output is contiguous
    AGC = 4
    rowb = np.array([(i * NT // AGC) * 128 for i in range(AGC + 1)])
    cumout = np.concatenate([[0], np.cumsum(NC * np.diff(rowb))])
    ls = src % NSH
    ci = np.searchsorted(rowb, ls, side='right') - 1
    trow = (cumout[ci] + (src // NSH) * (rowb[ci + 1] - rowb[ci])
            + (ls - rowb[ci]))
    g = (trow >= HALF).astype(np.int64)           # table half by row
    lidx = trow - g * HALF                        # int16-safe local row

    key = ((core * NT + t) * 2 + g)   # bucket id, core-major
    nbuck = NC * NT * 2
    cnt = np.bincount(key, minlength=nbuck).reshape(NC, NT, 2)
    CH = np.maximum(1, (cnt.max(axis=0) + 127) // 128)   # [NT, 2] shared chunks
    CH = CH + (CH & 1)   # even counts -> 4B-aligned stream slice offsets
    CHUNKS = int(CH.sum())
    choff = np.zeros((NT, 2), np.int64)
    choff.reshape(-1)[1:] = np.cumsum(CH.reshape(-1))[:-1]

    # stable-sort edges by bucket; ranks within bucket
    order = np.argsort(key, kind='stable')
    skey = key[order]
    bstart = np.searchsorted(skey, np.arange(nbuck))
    rank = np.arange(EE, dtype=np.int64) - bstart[skey]
    # padded stream position (per core stream of CHUNKS*128 slots)
    bt = (skey // 2) % NT
    bg = skey % 2
    pos = choff[bt, bg] * 128 + rank
    scor = skey // (NT * 2)

    gidx_all = np.zeros((NC, CHUNKS * 128), np.int16)
    dcol_all = np.full((NC, CHUNKS * 128), 999.0, np.float16)
    for c in range(NC):
        m = scor == c
        eidx = order[m]
        gidx_all[c, pos[m]] = lidx[eidx].astype(np.int16)
        dcol_all[c, pos[m]] = dit[eidx].astype(np.float16)

    # wrap: element i -> [i % 16, i // 16] / dstcol: chunk-major -> [128, CHUNKS]
    gidx = np.tile(gidx_all.reshape(NC, CHUNKS * 8, 16).transpose(0, 2, 1), (1, 8, 1)).copy()
    dcol = dcol_all.reshape(NC, CHUNKS, 128).transpose(0, 2, 1).copy()

    # batch / pooling (note: node n' holds old node inv[n'])
    batch = np.asarray(batch, np.int64)[inv]
    bcol = np.full((NC, 128, NT), 999.0, np.float16)
    for c in range(NC):
        bc_ = batch[c * NSH:(c + 1) * NSH].astype(np.float16)
        pad = np.full(NPAD - NSH, 999.0, np.float16)
        bcol[c] = np.concatenate([bc_, pad]).reshape(NT, 128).T
    cnt_g = np.bincount(batch, minlength=G).astype(f32)
    cntrecip = (1.0 / np.maximum(cnt_g, 1.0)).reshape(G, 1)

    # folded weights
    W1 = np.asarray(W1, f32); W2 = np.asarray(W2, f32)
    s1 = np.asarray(bn1_g, f32) / np.sqrt(np.asarray(bn1_v, f32) + BN_EPS)
    t1 = (np.asarray(bias1, f32) - np.asarray(bn1_m, f32)) * s1 + np.asarray(bn1_b, f32)
    s2 = np.asarray(bn2_g, f32) / np.sqrt(np.asarray(bn2_v, f32) + BN_EPS)
    t2 = (np.asarray(bias2, f32) - np.asarray(bn2_m, f32)) * s2 + np.asarray(bn2_b, f32)
    aS1 = np.asarray(att_src1, f32)   # [H, HID]
    aD1 = np.asarray(att_dst1, f32)
    # per-head rotation Q (first column = att_src) and inverse R = Q^-1 * s1
    rng = np.random.default_rng(12345)
    Qblk = np.zeros((H * HID, H * HID), f32)
    Rpack = np.zeros((128, 2, 128), f32)     # blockdiag pairs of R_h
    for h in range(H):
        a = aS1[h]
        M = np.concatenate([a[:, None],
                            rng.standard_normal((HID, HID - 1)).astype(f32)], 1)
        Qf, _ = np.linalg.qr(M)
        Q = np.concatenate([a[:, None], Qf[:, 1:]], 1).astype(f32)
        R = np.linalg.inv(Q).astype(f32) * s1[h * HID:(h + 1) * HID][None, :]
        Qblk[h * HID:(h + 1) * HID, h * HID:(h + 1) * HID] = Q
        b2, r2_ = divmod(h, 2)
        Rpack[r2_ * HID:(r2_ + 1) * HID, b2, r2_ * HID:(r2_ + 1) * HID] = R
    AblkD = np.zeros((H * HID, H), f32)
    for h in range(H):
        AblkD[h * HID:(h + 1) * HID, h] = aD1[h]
    W1e = np.concatenate([W1 @ Qblk, W1 @ AblkD], axis=1)            # [128, 260]
    t1colT = t1.reshape(2, 128).T.copy()                             # [128, 2]
    aS2 = np.asarray(att_src2, f32).reshape(HID)
    aD2 = np.asarray(att_dst2, f32).reshape(HID)
    W2e = np.concatenate([W2 * s2[None, :], (W2 @ aS2)[:, None],
                          (W2 @ aD2)[:, None]], axis=1)              # [256, 66]
    t2row = np.concatenate([t2, np.zeros(2, f32)]).reshape(1, 66)

    iotam = np.tile(np.arange(128, dtype=np.float16), (128, 1))
    iota64 = np.tile(np.arange(64, dtype=np.float16), (128, 1))
    ident = np.eye(128, dtype=f32)
    identh = np.eye(128, dtype=np.float16)
    onesrow = np.ones((1, 128), f32)

    x = np.asarray(x, f32)[inv]
    in_maps = []
    for c in range(NC):
        xs = x[c * NSH:(c + 1) * NSH]
        xT = np.zeros((DIN, NPAD), np.float16)
        xT[:, :NSH] = xs.T.astype(np.float16)
        in_maps.append({
            "xT": xT, "gidx": gidx[c], "dcol": dcol[c],
            "bcol": bcol[c].copy(),
            "W1e": W1e.astype(np.float16), "t1colT": t1colT,
            "Rpack": Rpack.astype(np.float16),
            "W2e": W2e.reshape(2, 128, 66).transpose(1, 0, 2).astype(np.float16).copy(),
            "t2row": t2row,
            "iotam": iotam, "iota64": iota64, "ident": ident, "identh": identh,
            "onesrow": onesrow,
            "Wc1": np.asarray(Wc1, f32), "bc1row": np.asarray(bc1, f32).reshape(1, HID),
            "Wc2": np.asarray(Wc2, f32), "bc2row": np.asarray(bc2, f32).reshape(1, 2),
            "cntrecip": cntrecip,
        })
    return in_maps, CH, choff, CHUNKS


def _build(CH, choff, CHUNKS):
    AluOp = mybir.AluOpType
    Act = mybir.ActivationFunctionType
    nc = bacc.Bacc("TRN2", target_bir_lowering=False, debug=False, num_devices=NC,
                   num_swdge_queues=4)

    xT_d = nc.dram_tensor("xT", [DIN, NPAD], dt.float16, kind="ExternalInput")
    gidx_d = nc.dram_tensor("gidx", [128, CHUNKS * 8], dt.int16, kind="ExternalInput")
    dcol_d = nc.dram_tensor("dcol", [128, CHUNKS], dt.float16, kind="ExternalInput")
    bcol_d = nc.dram_tensor("bcol", [128, NT], dt.float16, kind="ExternalInput")
    W1e_d = nc.dram_tensor("W1e", [DIN, 260], dt.float16, kind="ExternalInput")
    t1colT_d = nc.dram_tensor("t1colT", [128, 2], dt.float32, kind="ExternalInput")
    Rpack_d = nc.dram_tensor("Rpack", [128, 2, 128], dt.float16, kind="ExternalInput")
    W2e_d = nc.dram_tensor("W2e", [128, 2, 66], dt.float16, kind="ExternalInput")
    t2row_d = nc.dram_tensor("t2row", [1, 66], dt.float32, kind="ExternalInput")
    iotam_d = nc.dram_tensor("iotam", [128, 128], dt.float16, kind="ExternalInput")
    iota64_d = nc.dram_tensor("iota64", [128, 64], dt.float16, kind="ExternalInput")
    ident_d = nc.dram_tensor("ident", [128, 128], dt.float32, kind="ExternalInput")
    identh_d = nc.dram_tensor("identh", [128, 128], dt.float16, kind="ExternalInput")
    ones_d = nc.dram_tensor("onesrow", [1, 128], dt.float32, kind="ExternalInput")
    Wc1_d = nc.dram_tensor("Wc1", [HID, HID], dt.float32, kind="ExternalInput")
    bc1_d = nc.dram_tensor("bc1row", [1, HID], dt.float32, kind="ExternalInput")
    Wc2_d = nc.dram_tensor("Wc2", [HID, 2], dt.float32, kind="ExternalInput")
    bc2_d = nc.dram_tensor("bc2row", [1, 2], dt.float32, kind="ExternalInput")
    crec_d = nc.dram_tensor("cntrecip", [G, 1], dt.float32, kind="ExternalInput")
    out_d = nc.dram_tensor("out", [G, 2], dt.float32, kind="ExternalOutput")

    RG = [list(range(NC))]
    AGC = 4                      # allgather chunks per table
    bounds = [(i * NT // AGC) * 128 for i in range(AGC + 1)]

    with tile.TileContext(nc) as tc:
        with (
            tc.tile_pool(name="const", bufs=1) as cp,
            tc.tile_pool(name="sb", bufs=3) as sb,
            tc.tile_pool(name="gbuf", bufs=3) as gp,
            tc.tile_pool(name="gbuf2", bufs=3) as gp2,
            tc.tile_pool(name="small", bufs=6) as sp,
            tc.tile_pool(name="ps", bufs=2, space="PSUM") as ps,
            tc.tile_pool(name="pspool", bufs=1, space="PSUM") as psp,
            tc.tile_pool(name="dram", bufs=1, space="DRAM") as dram,
        ):
            nc.gpsimd.load_library(library_config.mlp)

            # ---- consts to SBUF
            def cload(dten, shape, dtype):
                tl = cp.tile(shape, dtype, tag=dten.name)
                nc.sync.dma_start(tl[:], dten[:])
                return tl
            W1e = cload(W1e_d, [DIN, 260], dt.float16)
            t1colT = cload(t1colT_d, [128, 2], dt.float32)
            Rpack = cload(Rpack_d, [128, 2, 128], dt.float16)
            W2e = cload(W2e_d, [128, 2, 66], dt.float16)
            t2row = cload(t2row_d, [1, 66], dt.float32)
            iotam = cload(iotam_d, [128, 128], dt.float16)
            iota64 = cload(iota64_d, [128, 64], dt.float16)
            ident = cload(ident_d, [128, 128], dt.float32)
            identh = cload(identh_d, [128, 128], dt.float16)
            ones = cload(ones_d, [1, 128], dt.float32)
            Wc1 = cload(Wc1_d, [HID, HID], dt.float32)
            bc1row = cload(bc1_d, [1, HID], dt.float32)
            Wc2 = cload(Wc2_d, [HID, 2], dt.float32)
            bc2row = cload(bc2_d, [1, 2], dt.float32)
            cntrecip = cload(crec_d, [G, 1], dt.float32)
            gidx = cload(gidx_d, [128, CHUNKS * 8], dt.int16)
            dcol = cload(dcol_d, [128, CHUNKS], dt.float16)
            bcol = cload(bcol_d, [128, NT], dt.float16)

            ad1 = cp.tile([128, NT, H], dt.float32, tag="ad1")
            ad2 = cp.tile([128, NT, 1], dt.float32, tag="ad2")
            poh = cp.tile([128, NT, G], dt.float16, tag="poh")
            # pooling one-hot (built once)
            nc.vector.tensor_tensor(
                poh[:],
                iota64[:].unsqueeze(1).broadcast_to([128, NT, G]),
                bcol[:].unsqueeze(2).broadcast_to([128, NT, G]),
                AluOp.is_equal)

            # ---- DRAM tables (collective outputs in Shared space)
            t1stage = dram.tile([NPAD, RF1], dt.float16)
            t1full = nc.dram_tensor("t1full", [NC * NPAD, RF1], dt.float16,
                                    kind="Internal", addr_space="Shared").ap()
            t2stage = dram.tile([NPAD, RF2], dt.float16)
            t2full = nc.dram_tensor("t2full", [NC * NPAD, RF2], dt.float16,
                                    kind="Internal", addr_space="Shared").ap()

            def ag_chunk(stage, full, ci):
                r0, r1 = bounds[ci], bounds[ci + 1]
                o0, o1 = NC * r0, NC * r1
                nc.gpsimd.collective_compute(
                    "AllGather", mybir.AluOpType.bypass, replica_groups=RG,
                    ins=[stage[r0:r1, :].opt()],
                    outs=[full[o0:o1, :].opt()])

            # ================= PHASE A: L1 head (y table + alpha_dst) =====
            with nc.named_scope("phaseA"), tc.tile_pool(name="head", bufs=3) as hp:
                ci = 0
                for t in range(NT):
                    xTt = hp.tile([DIN, 128], dt.float16, tag="xTt")
                    nc.sync.dma_start(xTt[:], xT_d[:, t * 128:(t + 1) * 128])
                    pa = ps.tile([128, 512], dt.float32, tag="pep")
                    nc.tensor.matmul(pa[:, 0:260], xTt[:], W1e[:],
                                     start=True, stop=True)
                    tab = sb.tile([128, RF1], dt.float16, tag="tab1")
                    nc.scalar.activation(tab[:], pa[:, 0:256], Act.Copy)
                    nc.vector.tensor_copy(ad1[:, t, :], pa[:, 256:260])
                    nc.sync.dma_start(t1stage[t * 128:(t + 1) * 128, :], tab[:])
                    if (t + 1) * 128 == bounds[ci + 1]:
                        ag_chunk(t1stage, t1full, ci)
                        ci += 1

            # ================= PHASE B: L1 edges + L2 head ================
            def edge_phase(layer, tfull, rfw, nh, adt, adrow_tag):
                """One GAT edge phase. Yields per-tile (t, hsb-or-h1T)."""
                halves = (tfull[0:HALF, :], tfull[HALF:2 * HALF, :])
                ncol = nh * HID   # message feature cols (256 / 64)
                rot = (layer == 1)
                maxc = int((CH[:, 0] + CH[:, 1]).max())
                for t in range(NT):
                    ct0 = int(choff[t, 0]); n0 = int(CH[t, 0])
                    ct1 = int(choff[t, 1]); n1 = int(CH[t, 1])
                    ctot = n0 + n1
                    adh = sp.tile([128, nh], dt.float16, tag=adrow_tag + "h")
                    nc.scalar.activation(adh[:], adt[:, t, :], Act.Copy)

                    gb = gp.tile([128, int(CH[:, 0].max() + CH[:, 1].max()), rfw],
                                 dt.float16, tag=f"gb{layer}")
                    for gi, (hoff, nch) in enumerate(((ct0, n0), (ct1, n1))):
                        boff = 0 if gi == 0 else n0
                        nc.gpsimd.dma_gather(
                            gb[:, boff:boff + nch, :], halves[gi],
                            gidx[:, hoff * 8:(hoff + nch) * 8],
                            num_idxs=nch * 128, num_idxs_reg=nch * 128,
                            elem_size=rfw, queue_num=(t * 2 + gi) % 4,
                            single_packet=(nch * 128 <= 1024))
                    # one-hot for all chunks of this tile
                    oh = gp2.tile([128, maxc, 128], dt.float16, tag="oh")
                    dc = dcol[:, ct0:ct0 + ctot]  # groups contiguous per tile
                    nc.vector.tensor_tensor(
                        oh[:, 0:ctot, :],
                        iotam[:].unsqueeze(1).broadcast_to([128, ctot, 128]),
                        dc.unsqueeze(2).broadcast_to([128, ctot, 128]),
                        AluOp.is_equal)
                    # shared psum bank per tile: [agg 0:ncol+nh | ade tail]
                    pbk = ps.tile([128, 440], dt.float32, tag="pagg")
                    pb = pbk[:, 0:ncol + nh]
                    ade = pbk[:, ncol + nh:ncol + nh + maxc * nh].rearrange(
                        "p (c k) -> p c k", k=nh)
                    # one-hot transpose on the PE (8 chunks per psum bank),
                    # copied to SBUF for the alpha_dst matmuls
                    ohT = gp2.tile([128, maxc * 128], dt.float16, tag="ohT")
                    for b0 in range(0, ctot, 8):
                        bn = min(8, ctot - b0)
                        ott = ps.tile([128, 512], dt.float32, tag="ptt")
                        oth = ott[:].bitcast(dt.float16)
                        for j in range(bn):
                            nc.tensor.transpose(oth[:, j * 128:(j + 1) * 128],
                                                oh[:, b0 + j, :], identh[:])
                        nc.scalar.copy(ohT[:, b0 * 128:(b0 + bn) * 128],
                                       oth[:, 0:bn * 128])
                    # alpha_dst per edge: ohT.T @ adh per chunk into psum strip
                    for c in range(ctot):
                        nc.tensor.matmul(
                            ade[:, c, :],
                            ohT[:, c * 128:(c + 1) * 128], adh[:],
                            start=True, stop=True)
                    # e = a_src + a_dst ; lrelu ; exp
                    ee = sp.tile([128, maxc, nh], dt.float32, tag=f"ee{layer}")
                    if rot:
                        as_ap = (gb[:, 0:ctot, :]
                                 .rearrange("p c (h f) -> p c h f", h=nh)
                                 [:, :, :, 0:1].squeeze(3))
                    else:
                        as_ap = gb[:, 0:ctot, ncol:ncol + 2 * nh].bitcast(dt.float32)
                    nc.vector.tensor_tensor(ee[:, 0:ctot, :], as_ap,
                                            ade[:, 0:ctot, :], AluOp.add)
                    nc.vector.scalar_tensor_tensor(
                        ee[:, 0:ctot, :], ee[:, 0:ctot, :], NEG_SLOPE,
                        ee[:, 0:ctot, :], AluOp.mult, AluOp.max)
                    ex = sp.tile([128, maxc, nh], dt.float16, tag=f"ex{layer}")
                    nc.scalar.activation(ex[:, 0:ctot, :], ee[:, 0:ctot, :],
                                         Act.Exp)
                    # scale messages by exp (in place, per head block);
                    # flattened 3D view keeps the access pattern simple
                    gb3 = gb[:, 0:ctot, 0:ncol].rearrange(
                        "p c (h f) -> p (c h) f", h=nh)
                    nc.vector.tensor_tensor(
                        gb3, gb3,
                        ex[:, 0:ctot, :].rearrange("p c h -> p (c h)")
                          .unsqueeze(2).broadcast_to([128, ctot * nh, HID]),
                        AluOp.mult)
                    # aggregate: messages and exp-sums into one psum bank
                    for c in range(ctot):
                        nc.tensor.matmul(pb[:, 0:ncol], oh[:, c, :],
                                         gb[:, c, 0:ncol],
                                         start=(c == 0), stop=(c == ctot - 1))
                        nc.tensor.matmul(pb[:, ncol:ncol + nh], oh[:, c, :],
                                         ex[:, c, :],
                                         start=(c == 0), stop=(c == ctot - 1))
                    # epilogue: h = num / (den + eps), relu (+ unrotation L1)
                    den = sp.tile([128, nh], dt.float32, tag=f"den{layer}")
                    nc.vector.tensor_scalar(den[:], pb[:, ncol:ncol + nh],
                                            1e-16, None, AluOp.add)
                    rec = sp.tile([128, nh], dt.float32, tag=f"rec{layer}")
                    nc.vector.reciprocal(rec[:], den[:])
                    if not rot:
                        hsb = sb.tile([128, ncol], dt.float16, tag=f"h{layer}")
                        nc.scalar.activation(hsb[:], pb[:, 0:ncol], Act.Relu,
                                             scale=rec[:, 0:1])
                        yield t, hsb
                        continue
                    hy = sb.tile([128, ncol], dt.float32, tag="hy")
                    for h in range(nh):
                        nc.scalar.activation(
                            hy[:, h * HID:(h + 1) * HID],
                            pb[:, h * HID:(h + 1) * HID], Act.Copy,
                            scale=rec[:, h:h + 1])
                    # transpose agg_y, unrotate (R = Q^-1 s1), bias+relu -> h1T
                    yT = sb.tile([128, 2, 128], dt.float16, tag="yT")
                    for k in range(2):
                        pt = ps.tile([128, 512], dt.float32, tag="pep")
                        nc.tensor.transpose(pt[:, 0:128],
                                            hy[:, k * 128:(k + 1) * 128],
                                            ident[:])
                        nc.scalar.copy(yT[:, k, :], pt[:, 0:128])
                    h1T = sb.tile([128, 2, 128], dt.float16, tag="h1T")
                    for k in range(2):
                        p2 = ps.tile([128, 512], dt.float32, tag="pep")
                        nc.tensor.matmul(p2[:, 0:128], Rpack[:, k, :],
                                         yT[:, k, :],
                                         start=True, stop=True)
                        nc.scalar.activation(h1T[:, k, :], p2[:, 0:128],
                                             Act.Relu, bias=t1colT[:, k:k + 1])
                    yield t, h1T

            # L1 edge phase; fused L2 head per tile
            ci = 0
            for t, h1T in edge_phase(1, t1full, RF1, H, ad1, "a1"):
                pc = ps.tile([128, 512], dt.float32, tag="pep")
                for k in range(2):
                    nc.tensor.matmul(pc[:, 0:66], h1T[:, k, :], W2e[:, k, :],
                                     start=(k == 0), stop=False)
                nc.tensor.matmul(pc[:, 0:66], ones[0:1, :], t2row[:],
                                 start=False, stop=True)
                tab2 = sb.tile([128, RF2], dt.float16, tag="tab2")
                nc.scalar.activation(tab2[:, 0:HID], pc[:, 0:HID], Act.Copy)
                nc.vector.tensor_copy(tab2[:, HID:HID + 2].bitcast(dt.float32),
                                      pc[:, HID:HID + 1])
                nc.vector.tensor_copy(ad2[:, t, :], pc[:, HID + 1:HID + 2])
                nc.vector.memset(tab2[:, HID + 2:RF2], 0.0)
                nc.sync.dma_start(t2stage[t * 128:(t + 1) * 128, :], tab2[:])
                if (t + 1) * 128 == bounds[ci + 1]:
                    ag_chunk(t2stage, t2full, ci)
                    ci += 1

            # ================= PHASE C: L2 edges + pooling ================
            pgsum = psp.tile([G, HID], dt.float32, tag="pgsum")
            for t, h2 in edge_phase(2, t2full, RF2, 1, ad2, "a2"):
                nc.tensor.matmul(pgsum[:], poh[:, t, :], h2[:],
                                 start=(t == 0), stop=(t == NT - 1))

            # ================= PHASE D: AllReduce + classifier ============
            ar_in = dram.tile([G, HID], dt.float32)
            ar_out = nc.dram_tensor("ar_out", [G, HID], dt.float32,
                                    kind="Internal", addr_space="Shared").ap()
            psum_sb = sb.tile([G, HID], dt.float32, tag="psum_sb")
            nc.vector.tensor_copy(psum_sb[:], pgsum[:])
            nc.sync.dma_start(ar_in[:], psum_sb[:])
            nc.gpsimd.collective_compute(
                "AllReduce", mybir.AluOpType.add, replica_groups=RG,
                ins=[ar_in.opt()], outs=[ar_out.opt()])
            rep = sb.tile([G, HID], dt.float32, tag="rep")
            nc.sync.dma_start(rep[:], ar_out[:])
            nc.vector.tensor_scalar(rep[:], rep[:], cntrecip[:, 0:1], None,
                                    AluOp.mult)
            # hc = relu(rep @ Wc1 + bc1)
            ptr = ps.tile([128, 512], dt.float32, tag="pep")
            nc.tensor.transpose(ptr[0:G, 0:G], rep[:], ident[0:G, 0:G])
            repT = sb.tile([G, G], dt.float32, tag="repT")
            nc.scalar.copy(repT[:], ptr[0:G, 0:G])
            ph = ps.tile([128, 512], dt.float32, tag="pep")
            nc.tensor.matmul(ph[0:G, 0:HID], repT[:], Wc1[:], start=True, stop=False)
            nc.tensor.matmul(ph[0:G, 0:HID], ones[0:1, 0:G], bc1row[:],
                             start=False, stop=True)
            hc = sb.tile([G, HID], dt.float32, tag="hc")
            nc.scalar.activation(hc[:], ph[0:G, 0:HID], Act.Relu)
            pt2 = ps.tile([128, 512], dt.float32, tag="pep")
            nc.tensor.transpose(pt2[0:G, 0:G], hc[:], ident[0:G, 0:G])
            hcT = sb.tile([G, G], dt.float32, tag="hcT")
            nc.scalar.copy(hcT[:], pt2[0:G, 0:G])
            pl = ps.tile([128, 512], dt.float32, tag="pep")
            nc.tensor.matmul(pl[0:G, 0:2], hcT[:], Wc2[:], start=True, stop=False)
            nc.tensor.matmul(pl[0:G, 0:2], ones[0:1, 0:G], bc2row[:],
                             start=False, stop=True)
            # log softmax over the 2 logits
            lg = sb.tile([G, 2], dt.float32, tag="lg")
            nc.vector.tensor_copy(lg[:], pl[0:G, 0:2])
            mx = sb.tile([G, 1], dt.float32, tag="mx")
            nc.vector.tensor_reduce(mx[:], lg[:], mybir.AxisListType.X,
                                    AluOp.max)
            nc.vector.tensor_scalar(lg[:], lg[:], mx[:, 0:1], None,
                                    AluOp.subtract)
            ex = sb.tile([G, 2], dt.float32, tag="ex")
            nc.scalar.activation(ex[:], lg[:], Act.Exp)
            sm = sb.tile([G, 1], dt.float32, tag="sm")
            nc.vector.tensor_reduce(sm[:], ex[:], mybir.AxisListType.X,
                                    AluOp.add)
            ls = sb.tile([G, 1], dt.float32, tag="ls")
            nc.scalar.activation(ls[:], sm[:], Act.Ln)
            outv = sb.tile([G, 2], dt.float32, tag="outv")
            nc.vector.tensor_scalar(outv[:], lg[:], ls[:, 0:1], None,
                                    AluOp.subtract)
            nc.sync.dma_start(out_d[:], outv[:])

    nc.compile()
    return nc


_last_result = [None]


def kernel(**inputs):
    import hashlib
    configure(int(np.asarray(inputs["x"]).shape[0]),
              int(np.asarray(inputs["edge_index"]).shape[1]))
    ek = np.ascontiguousarray(np.asarray(inputs["edge_index"]))
    bk = np.ascontiguousarray(np.asarray(inputs["batch"]))
    key = hashlib.sha1(ek.tobytes() + bk.tobytes()).hexdigest()
    in_maps, CH, choff, CHUNKS = _prep_host(**inputs)
    if key not in _cache:
        _cache[key] = _build(CH, choff, CHUNKS)
    nc = _cache[key]
    res = bass_utils.run_bass_kernel_spmd(nc, in_maps, core_ids=list(range(NC)))
    _last_result[0] = res
    return res.results[0]["out"].astype(np.float32)


def kernel_exec_ns():
    r = _last_result[0]
    return None if r is None else r.exec_time_ns
